# revision 65
# baseline (speedup 1.0000x reference)
"""Trainium2 Bass kernel for masked-biaffine BERT self-attention.

Data-parallel over batch (16 batches / 8 cores = 2 per core). Scores are
computed transposed (S[j,i], keys on partitions) so the additive
attention_mask is a per-partition exp bias and softmax normalization comes
free from a ones-column in the V matmul.

Per (b,h,jc) iteration:
  - q/k projections pair two heads per matmul ([128,512] PSUM tiles);
  - the five mask applications (ss+ab)*m split across engines: structs 0-2
    as DVE STTs straight from PSUM, structs 3-4 via an ACT bias-copy to
    SBUF (+ab rides the copy) followed by a Pool tensor_mul;
  - structs 0-1 and the DVE-summed 2-4 join the S0 PSUM bank through three
    PE identity matmuls, so exp reads PSUM directly;
  - context accumulates in PSUM with a ones-column denominator row, is
    copied once to SBUF f16 and DMA'd out; the transpose and
    1/denominator normalization happen on the host.
Work for batch 1 and later head-pairs drips through earlier iteration
slots (the engines are in-order, so emission order is the schedule).
1/sqrt(D) is folded into Wq/bq/abs_bias on the host; exp runs with scale=1.
"""

import sys

if "/opt/trn_rl_repo" not in sys.path:
    sys.path.insert(0, "/opt/trn_rl_repo")

import json

import numpy as np

import concourse.bass as bass
import concourse.mybir as mybir
import concourse.tile as tile
from concourse.masks import make_identity
from concourse.bass_utils import run_bass_kernel_spmd

# ---- BIR post-pass: this walrus build allows only one sync_info.on_wait ----
# entry per instruction; hoist extras onto inserted NoOps on the same engine.
_MAXW = 1
_split_ctr = [0]


def _split_waits_json(j):
    nsplit = 0
    for fn in j.get("functions", []):
        for blk in fn.get("blocks", []):
            out = []
            for body in blk.get("instructions", []):
                si = body.get("sync_info") or {}
                ow = si.get("on_wait") or []
                if len(ow) > _MAXW:
                    extra = ow[:-_MAXW]
                    si["on_wait"] = ow[-_MAXW:]
                    while extra:
                        grp, extra = extra[:_MAXW], extra[_MAXW:]
                        _split_ctr[0] += 1
                        out.append({
                            "debug": body.get("debug", 0),
                            "engine": body["engine"],
                            "ins": [],
                            "name": f"I-waitsplit-{_split_ctr[0]}",
                            "opcode": "NoOp",
                            "outs": [],
                            "sync_info": {"on_update": [], "on_wait": grp},
                        })
                    nsplit += 1
                out.append(body)
            blk["instructions"] = out
    return nsplit


def _install_birfix():
    import concourse.bass_utils as bu
    import concourse.bass2jax as b2j

    if getattr(bu, "_waitsplit_installed", False):
        return
    orig = bu.compile_bir_kernel

    def patched(bir_json, tmpdir, neff_name="file.neff"):
        j = json.loads(bir_json)
        _split_waits_json(j)
        return orig(json.dumps(j).encode(), tmpdir, neff_name)

    bu.compile_bir_kernel = patched
    b2j.compile_bir_kernel = patched
    bu._waitsplit_installed = True


_install_birfix()

B, L, HID, H, D = 16, 512, 768, 12, 64
NS = 5
NCORES = 8
NB = B // NCORES          # batches per core
TOK = NB * L              # tokens per core
NJC = L // 128            # j-chunks per (b,h)
NG = HID // 128           # head-pair groups (6)
F32 = mybir.dt.float32
F16 = mybir.dt.float16
AF = mybir.ActivationFunctionType
OP = mybir.AluOpType

# struct -> mask-apply path. GPSIMD cannot read PSUM (and only supports
# plain TensorTensor ops), so:
#   'd' = DVE STT straight from PSUM;
#   'a' = ACT copy PSUM->SBUF f16 (the +ab bias rides the copy), then the
#         mask-multiply runs on Pool as tensor_mul from SBUF.
STT_ENG = ['d', 'd', 'd', 'a', 'a']
IDS = [0, 1]        # structs identity-accumulated on PE into the S0 bank
TREE = [2, 3, 4]    # summed by two DVE adds; joins s0 via a third identity

LAST_RESULT = None  # BassKernelResults of the most recent run (for test.py)


def _build_nc():
    nc = bass.Bass()

    # ---- DRAM I/O (per core) ----
    xt_h = nc.dram_tensor("xt", [HID, TOK], F16, kind="ExternalInput")
    wqt_h = nc.dram_tensor("wqt", [HID, HID], F16, kind="ExternalInput")
    wkt_h = nc.dram_tensor("wkt", [HID, HID], F16, kind="ExternalInput")
    wvt_h = nc.dram_tensor("wvt", [HID, HID], F16, kind="ExternalInput")
    bqt_h = nc.dram_tensor("bqt", [128, NG], F32, kind="ExternalInput")
    bkt_h = nc.dram_tensor("bkt", [128, NG], F32, kind="ExternalInput")
    bv_h = nc.dram_tensor("bv", [HID], F32, kind="ExternalInput")
    bilir_h = nc.dram_tensor("bilir", [D, H, NS, D], F16, kind="ExternalInput")
    absb_h = nc.dram_tensor("absb", [NS * H], F32, kind="ExternalInput")
    amt_h = nc.dram_tensor("amt", [128, NB * NJC], F32, kind="ExternalInput")
    maskt_h = nc.dram_tensor("maskt", [NB, NJC, 128, NS, L], F16, kind="ExternalInput")
    # [b, h, 65, i]: rows 0..63 = unnormalized ctx^T, row 64 = softmax denom
    out_h = nc.dram_tensor("out", [NB, H, 65, L], F16, kind="ExternalOutput")

    with tile.TileContext(nc) as tc:
        with tc.tile_pool(name="pers", bufs=1) as pers:
            # persistent SBUF tensors
            # paired q/k: group g holds heads (2g, 2g+1) on partition halves
            qt_t = [pers.tile([128, NG, L], F16, tag=f"qt{b}", name=f"qt{b}")
                    for b in range(NB)]
            kt_t = [pers.tile([128, NG, L], F16, tag=f"kt{b}", name=f"kt{b}")
                    for b in range(NB)]
            v_t = [pers.tile([128, H * 65], F16, tag=f"v{ic}", name=f"v{ic}")
                   for ic in range(NB * NJC)]
            # both partition halves hold the same [d, h, s, p] data so lhsT
            # can start at partition 0 or 64 to match the head's parity
            bilir_sb = pers.tile([128, H, NS, D], F16, tag="bilir")
            absb_sb = pers.tile([128, NS * H], F32, tag="absb")
            amt_sb = pers.tile([128, NB * NJC], F32, tag="amt")
            bqt_sb = pers.tile([128, NG], F32, tag="bqt")
            bkt_sb = pers.tile([128, NG], F32, tag="bkt")
            bv_sb = pers.tile([128, HID], F32, tag="bv")
            ident = pers.tile([128, 128], F16, tag="ident")

            make_identity(nc, ident[:, :])
            nc.sync.dma_start(out=bilir_sb[0:64, :, :, :], in_=bilir_h[:, :, :, :])
            nc.sync.dma_start(out=bilir_sb[64:128, :, :, :], in_=bilir_h[:, :, :, :])
            nc.sync.dma_start(out=amt_sb[:, :], in_=amt_h[:, :])
            nc.sync.dma_start(out=bqt_sb[:, :], in_=bqt_h[:, :])
            nc.sync.dma_start(out=bkt_sb[:, :], in_=bkt_h[:, :])
            ab_ap = absb_h[:]
            nc.gpsimd.dma_start(
                out=absb_sb[:, :],
                in_=bass.AP(tensor=ab_ap.tensor, offset=ab_ap.offset,
                            ap=[[0, 128], [1, NS * H]]),
            )
            bv_ap = bv_h[:]
            nc.gpsimd.dma_start(
                out=bv_sb[:, :],
                in_=bass.AP(tensor=bv_ap.tensor, offset=bv_ap.offset,
                            ap=[[0, 128], [1, HID]]),
            )
            # ones columns of v_ext: preset whole tile to 1.0; projection
            # evacuations overwrite the 64 value columns of each head slot.
            for ic in range(NB * NJC):
                nc.vector.memset(v_t[ic][:, :], 1.0)

            with (
                tc.tile_pool(name="stageb", bufs=1) as stb,
                tc.tile_pool(name="mpool", bufs=2) as mpool,
                tc.tile_pool(name="att", bufs=2) as att,
                tc.tile_pool(name="sc_ps", bufs=1, space="PSUM") as scp,
                tc.tile_pool(name="ctx_ps", bufs=2, space="PSUM") as ctxp,
            ):
                xt_sb = stb.tile([128, NG, TOK], F16, tag="xt")
                wq_sb = stb.tile([128, NG, HID], F16, tag="wq")
                wk_sb = stb.tile([128, NG, HID], F16, tag="wk")
                wv_sb = stb.tile([128, NG, HID], F16, tag="wv")
                # masks for batch b, loaded once, reused by all 12 heads
                mk_b = [mpool.tile([128, NJC, NS, L], F16, tag="mask",
                                   name=f"mask{b}") for b in range(NB)]

                # DMA queue ordered along the first iteration's critical
                # path: q's operands, then the first mask chunk, then the
                # rest interleaved.
                for hc in range(NG):
                    nc.sync.dma_start(out=wq_sb[:, hc, :], in_=wqt_h[hc * 128:(hc + 1) * 128, :])
                    nc.sync.dma_start(out=xt_sb[:, hc, :], in_=xt_h[hc * 128:(hc + 1) * 128, :])
                nc.sync.dma_start(out=mk_b[0][:, 0, :, :], in_=maskt_h[0, 0, :, :, :])
                for hc in range(NG):
                    nc.sync.dma_start(out=wk_sb[:, hc, :], in_=wkt_h[hc * 128:(hc + 1) * 128, :])
                for hc in range(NG):
                    nc.sync.dma_start(out=wv_sb[:, hc, :], in_=wvt_h[hc * 128:(hc + 1) * 128, :])
                for jc in range(1, NJC):
                    nc.sync.dma_start(out=mk_b[0][:, jc, :, :], in_=maskt_h[0, jc, :, :, :])
                for jc in range(NJC):
                    nc.sync.dma_start(out=mk_b[1][:, jc, :, :], in_=maskt_h[1, jc, :, :, :])

                def emit_qk_unit(b, which, g):
                    # one head-pair group of the q or k projection
                    w_sb, t_sb, bias_sb = (
                        (wq_sb, qt_t[b], bqt_sb) if which == 'q'
                        else (wk_sb, kt_t[b], bkt_sb))
                    ps = scp.tile([128, L], F32, tag="s0", bufs=2,
                                  name=f"pj{which}{b}{g}")
                    for hc in range(NG):
                        nc.tensor.matmul(
                            ps[:, :],
                            lhsT=w_sb[:, hc, g * 128:(g + 1) * 128],
                            rhs=xt_sb[:, hc, b * L:(b + 1) * L],
                            start=(hc == 0), stop=(hc == NG - 1),
                        )
                    nc.scalar.activation(
                        t_sb[0:64, g, :], ps[0:64, :], AF.Identity,
                        bias=bias_sb[0:64, g:g + 1], scale=1.0,
                    )
                    nc.scalar.activation(
                        t_sb[64:128, g, :], ps[64:128, :], AF.Identity,
                        bias=bias_sb[64:128, g:g + 1], scale=1.0,
                    )

                def emit_v_unit(b, jc, ow, osz):
                    ic = b * NJC + jc
                    ps = scp.tile([128, 512], F32, tag="ss", bufs=4,
                                  name=f"vps{ic}{ow}")
                    for hc in range(NG):
                        nc.tensor.matmul(
                            ps[:, 0:osz],
                            lhsT=xt_sb[:, hc, ic * 128:(ic + 1) * 128],
                            rhs=wv_sb[:, hc, ow:ow + osz],
                            start=(hc == 0), stop=(hc == NG - 1),
                        )
                    h0 = ow // 64
                    nh = osz // 64
                    dst = v_t[ic][:, h0 * 65:(h0 + nh) * 65].rearrange(
                        "p (h e) -> p h e", e=65)[:, :, 0:64]
                    # DVE reads the PSUM into the slots; the host-broadcast
                    # bias is added there (Pool cannot read PSUM)
                    nc.vector.tensor_add(
                        dst,
                        ps[:, 0:osz].rearrange("p (h q) -> p h q", q=64),
                        bv_sb[:, ow:ow + osz].rearrange("p (h q) -> p h q", q=64),
                    )

                # minimal upfront (first three head-pairs + early V) so the
                # first attention iterations start ~30us sooner; the rest
                # drips through iteration slots ahead of use.
                for g in range(3):
                    emit_qk_unit(0, 'q', g)
                    emit_qk_unit(0, 'k', g)
                upfront_v = [(0, jc, ow, osz) for jc in range(2)
                             for ow, osz in ((0, 512), (512, 256))]
                def v_units(b, jc):
                    return [('v', b, jc, ow, osz)
                            for ow, osz in ((0, 512), (512, 256))]

                def qk_units(b, g):
                    return [('qk', b, w, g) for w in ('q', 'k')]

                # ordered so each unit lands before its first consumer
                drip_units = (
                    qk_units(0, 3) + v_units(0, 2) + v_units(0, 3)
                    + qk_units(0, 4) + qk_units(0, 5) + qk_units(1, 0)
                    + v_units(1, 0) + qk_units(1, 1) + v_units(1, 1)
                    + qk_units(1, 2) + v_units(1, 2) + qk_units(1, 3)
                    + qk_units(1, 4) + v_units(1, 3) + qk_units(1, 5)
                )

                def emit_drip(u):
                    if u[0] == 'qk':
                        emit_qk_unit(u[1], u[2], u[3])
                    else:
                        emit_v_unit(u[1], u[2], u[3], u[4])

                # ---- attention ----
                # Two heads (even/odd of each pair group) are software-
                # pipelined: their iteration bodies interleave so one
                # stream's matmul->STT->combine->exp chain latency hides
                # under the other stream's engine work.
                sgroups = [(0, 2), (2, 4), (4, 5)]

                qs_tiles = {}

                def emit_qs_unit(b, h, gi):
                    """One qs work unit: 1-2 matmuls + one PSUM evacuation.
                    Units are dripped through the schedule so the qs of pair
                    p+2 materializes during pair p's iterations."""
                    p0 = 64 * (h & 1)
                    g = h >> 1
                    qt_head = qt_t[b][p0:p0 + 64, g, :]
                    if (b, h) not in qs_tiles:
                        qs_tiles[(b, h)] = att.tile(
                            [128, NS, L], F16, tag="qs", bufs=6,
                            name=f"qs{b}_{h}")
                    qs_sb = qs_tiles[(b, h)]
                    s0i, s1i = sgroups[gi]
                    for s in range(s0i, s1i):
                        qs_ps = scp.tile([128, L], F32, tag="ss", bufs=4,
                                         name=f"qsps{b}{h}{s}")
                        nc.tensor.matmul(
                            qs_ps[p0:p0 + 64, :],
                            lhsT=bilir_sb[p0:p0 + 64, h, s, :],
                            rhs=qt_head,
                            start=True, stop=True,
                        )
                        if s == 4:  # spread evacuation load off ACT
                            nc.vector.tensor_copy(
                                qs_sb[p0:p0 + 64, s, :], qs_ps[p0:p0 + 64, :])
                        else:
                            nc.scalar.copy(
                                qs_sb[p0:p0 + 64, s, :], qs_ps[p0:p0 + 64, :])

                def emit_scores(b, h, jc, qs_sb):
                    """Phase 1: struct-score matmuls + mask STTs."""
                    p0 = 64 * (h & 1)
                    g = h >> 1
                    kt_j = kt_t[b][p0:p0 + 64, g, jc * 128:(jc + 1) * 128]
                    u_d = att.tile([128, 3, L], F16, tag="ud", bufs=4,
                                   name=f"ud{b}{h}{jc}")
                    u_p = att.tile([128, 2, L], F16, tag="up", bufs=4,
                                   name=f"up{b}{h}{jc}")
                    ssc = att.tile([128, 2, L], F16, tag="ssc", bufs=4,
                                   name=f"ssc{b}{h}{jc}")
                    nd = 0
                    npl = 0
                    uslot = {}
                    for s in range(NS):
                        ss = scp.tile([128, L], F32, tag="ss", bufs=4,
                                      name=f"ss{b}{h}{jc}{s}")
                        nc.tensor.matmul(
                            ss[:, :],
                            lhsT=kt_j,
                            rhs=qs_sb[p0:p0 + 64, s, :],
                            start=True, stop=True,
                        )
                        ab = absb_sb[:, s * H + h:s * H + h + 1]
                        mk = mk_b[b][:, jc, s, :]
                        if STT_ENG[s] == 'd':
                            dst = u_d[:, nd, :]
                            uslot[s] = dst
                            nd += 1
                            nc.vector.scalar_tensor_tensor(
                                dst, ss[:, :], ab, mk, OP.add, OP.mult)
                        else:
                            # ACT evacuates (ss + ab) to SBUF; Pool masks it
                            nc.scalar.activation(
                                ssc[:, npl, :], ss[:, :], AF.Identity,
                                bias=ab, scale=1.0)
                            dst = u_p[:, npl, :]
                            uslot[s] = dst
                            nc.gpsimd.tensor_mul(dst, ssc[:, npl, :], mk)
                            npl += 1
                    return uslot

                def emit_combine(b, h, jc, uslot, ctx_ps):
                    """Phase 2: S0 matmul, combines, exp, context matmul."""
                    p0 = 64 * (h & 1)
                    g = h >> 1
                    qt_head = qt_t[b][p0:p0 + 64, g, :]
                    kt_j = kt_t[b][p0:p0 + 64, g, jc * 128:(jc + 1) * 128]
                    # structs 2-4 summed on DVE (f16 2x adds)
                    ta = att.tile([128, L], F16, tag="ta", bufs=3,
                                  name=f"ta{b}{h}{jc}")
                    nc.vector.tensor_add(ta[:, :], uslot[TREE[0]],
                                         uslot[TREE[1]])
                    tb = att.tile([128, L], F16, tag="tb", bufs=3,
                                  name=f"tb{b}{h}{jc}")
                    nc.vector.tensor_add(tb[:, :], ta[:, :], uslot[TREE[2]])
                    s0 = scp.tile([128, L], F32, tag="s0", bufs=2,
                                  name=f"s0_{b}_{h}_{jc}")
                    nc.tensor.matmul(s0[:, :], lhsT=kt_j, rhs=qt_head,
                                     start=True, stop=False,
                                     skip_group_check=True)
                    # structs 0-2 + the pair sum: PE identities into s0
                    adds = [uslot[s] for s in IDS] + [tb[:, :]]
                    for i, rhs in enumerate(adds):
                        nc.tensor.matmul(s0[:, :], lhsT=ident[:, :],
                                         rhs=rhs,
                                         start=False,
                                         stop=(i == len(adds) - 1),
                                         skip_group_check=True)
                    pr = att.tile([128, L], F16, tag="pr", bufs=4,
                                  name=f"pr{b}{h}{jc}")
                    nc.scalar.activation(
                        pr[:, :], s0[:, :], AF.Exp,
                        bias=amt_sb[:, b * NJC + jc:b * NJC + jc + 1],
                        scale=1.0,
                    )
                    nc.tensor.matmul(
                        ctx_ps[0:65, :],
                        lhsT=v_t[b * NJC + jc][:, h * 65:(h + 1) * 65],
                        rhs=pr[:, :],
                        start=(jc == 0), stop=(jc == NJC - 1),
                        skip_group_check=True,
                    )

                pairs = [(b, g) for b in range(NB) for g in range(NG)]

                def qs_units(pi):
                    if pi >= len(pairs):
                        return []
                    b_, g_ = pairs[pi]
                    return [(b_, 2 * g_ + hi, gi)
                            for hi in range(2) for gi in range(3)]

                # pairs 0 and 1 cold-start; pair p+2's qs drips through
                # pair p's iteration slots.
                for unit in qs_units(0) + qs_units(1):
                    emit_qs_unit(*unit)
                # V(b0) early chunks after the qs cold-start so a wv DMA
                # wait cannot block the first score matmuls
                for u in upfront_v:
                    emit_v_unit(*u)
                for pi, (b, g) in enumerate(pairs):
                    heads = (2 * g, 2 * g + 1)
                    qs_pair = [qs_tiles[(b, h)] for h in heads]
                    drip = list(qs_units(pi + 2))
                    ctx_pair = [ctxp.tile([65, L], F32, tag="ctx",
                                          name=f"ctx{b}_{h}")
                                for h in heads]
                    for jc in range(NJC):
                        us = []
                        for hi, h in enumerate(heads):
                            us.append(emit_scores(b, h, jc, qs_pair[hi]))
                            if drip:
                                emit_qs_unit(*drip.pop(0))
                        for _ in range(2 if pi < 3 else 1):
                            if drip_units:
                                emit_drip(drip_units.pop(0))
                        for hi, h in enumerate(heads):
                            emit_combine(b, h, jc, us[hi], ctx_pair[hi])
                    # raw ctx^T + denominator row to DRAM via an SBUF
                    # bounce; the host transposes and normalizes.
                    for hi, h in enumerate(heads):
                        ct = att.tile([65, L], F16, tag="ct", bufs=4,
                                      name=f"ct{b}_{h}")
                        nc.scalar.copy(ct[:, :], ctx_pair[hi][0:65, :])
                        nc.sync.dma_start(out=out_h[b, h, :, :], in_=ct[:, :])
    return nc


_NC = None


def _get_nc():
    global _NC
    if _NC is None:
        _NC = _build_nc()
    return _NC


def _prep_inputs(inputs):
    hs = np.asarray(inputs["hidden_states"], np.float32)
    am = np.asarray(inputs["attention_mask"], np.float32).reshape(B, L)
    sm = np.asarray(inputs["structure_mask"], np.float32)
    Wq = np.asarray(inputs["Wq"], np.float32)
    Wk = np.asarray(inputs["Wk"], np.float32)
    Wv = np.asarray(inputs["Wv"], np.float32)
    bq = np.asarray(inputs["bq"], np.float32)
    bk = np.asarray(inputs["bk"], np.float32)
    bv = np.asarray(inputs["bv"], np.float32)
    bili = np.asarray(inputs["bili"], np.float32)
    absb = np.asarray(inputs["abs_bias"], np.float32)

    sc = np.float32(1.0 / np.sqrt(D))  # folded into the whole q side
    shared = {
        "wqt": np.ascontiguousarray(Wq.T * sc).astype(np.float16),
        "wkt": np.ascontiguousarray(Wk.T).astype(np.float16),
        "wvt": np.ascontiguousarray(Wv.T).astype(np.float16),
        "bqt": np.ascontiguousarray((bq * sc).reshape(NG, 128).T),
        "bkt": np.ascontiguousarray(bk.reshape(NG, 128).T),
        "bv": bv,
        # [d, h, s, p] so lhsT slice [:, h, s:s+2, :] pairs two structs
        "bilir": np.ascontiguousarray(
            bili.transpose(2, 1, 0, 3)).astype(np.float16),
        "absb": np.ascontiguousarray(absb.reshape(NS * H) * sc),
    }
    in_maps = []
    for c in range(NCORES):
        b0 = c * NB
        x = hs[b0:b0 + NB].reshape(TOK, HID)
        amc = am[b0:b0 + NB]  # [NB, L]
        # -10: constant logit shift (softmax-invariant) keeping exp() and the
        # row sums inside fp16 range without a max-reduction pass.
        amt = np.ascontiguousarray(
            amc.reshape(NB, NJC, 128).transpose(2, 0, 1)).reshape(128, NB * NJC) - 10.0
        mk = sm[:, b0:b0 + NB, 0]  # [NS, NB, L(i), L(j)]
        mkt = np.ascontiguousarray(mk.transpose(1, 3, 0, 2))  # [NB, j, NS, i]
        mkt = mkt.reshape(NB, NJC, 128, NS, L).astype(np.float16)
        in_maps.append(dict(
            xt=np.ascontiguousarray(x.T).astype(np.float16), amt=amt, maskt=mkt, **shared))
    return in_maps


def kernel(**inputs):
    global LAST_RESULT
    nc = _get_nc()
    in_maps = _prep_inputs(inputs)
    import os
    trace = bool(os.environ.get("BASS_TRACE"))
    try:
        LAST_RESULT = run_bass_kernel_spmd(
            nc, in_maps, core_ids=list(range(NCORES)), trace=trace)
    except ModuleNotFoundError:
        # axon NTFF profile hook unavailable in this environment
        LAST_RESULT = run_bass_kernel_spmd(
            nc, in_maps, core_ids=list(range(NCORES)), trace=False)
    outs = np.stack([r["out"] for r in LAST_RESULT.results]).astype(np.float32)
    den = outs[:, :, :, 64:65, :]  # [8, NB, H, 1, L]
    vals = outs[:, :, :, 0:64, :] / den
    # [c, b, h, d, i] -> [c, b, i, h, d] -> [B, L, HID]
    return np.ascontiguousarray(vals.transpose(0, 1, 4, 2, 3)).reshape(B, L, HID)


# revision 79
# speedup vs baseline: 1.0128x; 1.0128x over previous
"""Trainium2 Bass kernel for masked-biaffine BERT self-attention.

Data-parallel over batch (16 batches / 8 cores = 2 per core). Scores are
computed transposed (S[j,i], keys on partitions) so the additive
attention_mask is a per-partition exp bias and softmax normalization comes
free from a ones-column in the V matmul.

Per (b,h,jc) iteration:
  - q/k projections pair two heads per matmul ([128,512] PSUM tiles);
  - the five mask applications (ss+ab)*m split across engines: structs 0-2
    as DVE STTs straight from PSUM, structs 3-4 via an ACT bias-copy to
    SBUF (+ab rides the copy) followed by a Pool tensor_mul;
  - structs 0-1 and the DVE-summed 2-4 join the S0 PSUM bank through three
    PE identity matmuls, so exp reads PSUM directly;
  - context accumulates in PSUM with a ones-column denominator row, is
    copied once to SBUF f16 and DMA'd out; the transpose and
    1/denominator normalization happen on the host.
Work for batch 1 and later head-pairs drips through earlier iteration
slots (the engines are in-order, so emission order is the schedule).
1/sqrt(D) is folded into Wq/bq/abs_bias on the host; exp runs with scale=1.
"""

import sys

if "/opt/trn_rl_repo" not in sys.path:
    sys.path.insert(0, "/opt/trn_rl_repo")

import json

import numpy as np

import concourse.bass as bass
import concourse.mybir as mybir
import concourse.tile as tile
from concourse.masks import make_identity
from concourse.bass_utils import run_bass_kernel_spmd

# ---- BIR post-pass: this walrus build allows only one sync_info.on_wait ----
# entry per instruction; hoist extras onto inserted NoOps on the same engine.
_MAXW = 1
_split_ctr = [0]


def _split_waits_json(j):
    nsplit = 0
    for fn in j.get("functions", []):
        for blk in fn.get("blocks", []):
            out = []
            for body in blk.get("instructions", []):
                si = body.get("sync_info") or {}
                ow = si.get("on_wait") or []
                if len(ow) > _MAXW:
                    extra = ow[:-_MAXW]
                    si["on_wait"] = ow[-_MAXW:]
                    while extra:
                        grp, extra = extra[:_MAXW], extra[_MAXW:]
                        _split_ctr[0] += 1
                        out.append({
                            "debug": body.get("debug", 0),
                            "engine": body["engine"],
                            "ins": [],
                            "name": f"I-waitsplit-{_split_ctr[0]}",
                            "opcode": "NoOp",
                            "outs": [],
                            "sync_info": {"on_update": [], "on_wait": grp},
                        })
                    nsplit += 1
                out.append(body)
            blk["instructions"] = out
    return nsplit


def _install_birfix():
    import concourse.bass_utils as bu
    import concourse.bass2jax as b2j

    if getattr(bu, "_waitsplit_installed", False):
        return
    orig = bu.compile_bir_kernel

    def patched(bir_json, tmpdir, neff_name="file.neff"):
        j = json.loads(bir_json)
        _split_waits_json(j)
        return orig(json.dumps(j).encode(), tmpdir, neff_name)

    bu.compile_bir_kernel = patched
    b2j.compile_bir_kernel = patched
    bu._waitsplit_installed = True


_install_birfix()

B, L, HID, H, D = 16, 512, 768, 12, 64
NS = 5
NCORES = 8
NB = B // NCORES          # batches per core
TOK = NB * L              # tokens per core
NJC = L // 128            # j-chunks per (b,h)
NG = HID // 128           # head-pair groups (6)
F32 = mybir.dt.float32
F16 = mybir.dt.float16
AF = mybir.ActivationFunctionType
OP = mybir.AluOpType

# struct -> mask-apply path. GPSIMD cannot read PSUM (and only supports
# plain TensorTensor ops), so:
#   'd' = DVE STT straight from PSUM;
#   'a' = ACT copy PSUM->SBUF f16 (the +ab bias rides the copy), then the
#         mask-multiply runs on Pool as tensor_mul from SBUF.
STT_ENG = ['d', 'd', 'd', 'a', 'a']
IDS = [0, 1]        # structs identity-accumulated on PE into the S0 bank
TREE = [2, 3, 4]    # summed by two DVE adds; joins s0 via a third identity

LAST_RESULT = None  # BassKernelResults of the most recent run (for test.py)


def _build_nc():
    nc = bass.Bass()

    # ---- DRAM I/O (per core) ----
    xt_h = nc.dram_tensor("xt", [HID, TOK], F16, kind="ExternalInput")
    wqt_h = nc.dram_tensor("wqt", [HID, HID], F16, kind="ExternalInput")
    wkt_h = nc.dram_tensor("wkt", [HID, HID], F16, kind="ExternalInput")
    wvt_h = nc.dram_tensor("wvt", [HID, HID], F16, kind="ExternalInput")
    bqt_h = nc.dram_tensor("bqt", [128, NG], F32, kind="ExternalInput")
    bkt_h = nc.dram_tensor("bkt", [128, NG], F32, kind="ExternalInput")
    bv_h = nc.dram_tensor("bv", [HID], F32, kind="ExternalInput")
    bilir_h = nc.dram_tensor("bilir", [D, H, NS, D], F16, kind="ExternalInput")
    absb_h = nc.dram_tensor("absb", [NS * H], F32, kind="ExternalInput")
    amt_h = nc.dram_tensor("amt", [128, NB * NJC], F32, kind="ExternalInput")
    maskt_h = nc.dram_tensor("maskt", [NB, NJC, 128, NS, L], F16, kind="ExternalInput")
    # [b, h, 65, i]: rows 0..63 = unnormalized ctx^T, row 64 = softmax denom
    out_h = nc.dram_tensor("out", [NB, H, 65, L], F16, kind="ExternalOutput")

    with tile.TileContext(nc) as tc:
        with tc.tile_pool(name="pers", bufs=1) as pers:
            # persistent SBUF tensors
            # paired q/k: group g holds heads (2g, 2g+1) on partition halves
            qt_t = [pers.tile([128, NG, L], F16, tag=f"qt{b}", name=f"qt{b}")
                    for b in range(NB)]
            kt_t = [pers.tile([128, NG, L], F16, tag=f"kt{b}", name=f"kt{b}")
                    for b in range(NB)]
            v_t = [pers.tile([128, H * 65], F16, tag=f"v{ic}", name=f"v{ic}")
                   for ic in range(NB * NJC)]
            # both partition halves hold the same [d, h, s, p] data so lhsT
            # can start at partition 0 or 64 to match the head's parity
            bilir_sb = pers.tile([128, H, NS, D], F16, tag="bilir")
            absb_sb = pers.tile([128, NS * H], F32, tag="absb")
            amt_sb = pers.tile([128, NB * NJC], F32, tag="amt")
            bqt_sb = pers.tile([128, NG], F32, tag="bqt")
            bkt_sb = pers.tile([128, NG], F32, tag="bkt")
            bv_sb = pers.tile([128, HID], F32, tag="bv")
            ident = pers.tile([128, 128], F16, tag="ident")

            make_identity(nc, ident[:, :])
            nc.sync.dma_start(out=amt_sb[:, :], in_=amt_h[:, :])
            nc.sync.dma_start(out=bqt_sb[:, :], in_=bqt_h[:, :])
            nc.sync.dma_start(out=bkt_sb[:, :], in_=bkt_h[:, :])
            ab_ap = absb_h[:]
            nc.gpsimd.dma_start(
                out=absb_sb[:, :],
                in_=bass.AP(tensor=ab_ap.tensor, offset=ab_ap.offset,
                            ap=[[0, 128], [1, NS * H]]),
            )
            bv_ap = bv_h[:]
            nc.gpsimd.dma_start(
                out=bv_sb[:, :],
                in_=bass.AP(tensor=bv_ap.tensor, offset=bv_ap.offset,
                            ap=[[0, 128], [1, HID]]),
            )
            # ones columns of v_ext: preset whole tile to 1.0; projection
            # evacuations overwrite the 64 value columns of each head slot.
            for ic in range(NB * NJC):
                nc.vector.memset(v_t[ic][:, :], 1.0)

            with (
                tc.tile_pool(name="stageb", bufs=1) as stb,
                tc.tile_pool(name="mpool", bufs=2) as mpool,
                tc.tile_pool(name="att", bufs=2) as att,
                tc.tile_pool(name="sc_ps", bufs=1, space="PSUM") as scp,
                tc.tile_pool(name="ctx_ps", bufs=2, space="PSUM") as ctxp,
            ):
                xt_sb = stb.tile([128, NG, TOK], F16, tag="xt")
                wq_sb = stb.tile([128, NG, HID], F16, tag="wq")
                wk_sb = stb.tile([128, NG, HID], F16, tag="wk")
                wv_sb = stb.tile([128, NG, HID], F16, tag="wv")
                # masks for batch b, loaded once, reused by all 12 heads
                mk_b = [mpool.tile([128, NJC, NS, L], F16, tag="mask",
                                   name=f"mask{b}") for b in range(NB)]

                # DMA queue ordered along the first iteration's critical
                # path: q's operands, then the first mask chunk, then the
                # rest interleaved.
                # batch-0's x half first: the first projection matmuls gate
                # on (wq chunk, xt-b0 chunk) pairs
                # DMA queue ordered along the first iterations' critical
                # path: q then k operands, bilir (for qs), first mask
                # chunk, then the rest.
                for hc in range(NG):
                    nc.sync.dma_start(out=wq_sb[:, hc, :], in_=wqt_h[hc * 128:(hc + 1) * 128, :])
                    nc.sync.dma_start(out=xt_sb[:, hc, 0:L], in_=xt_h[hc * 128:(hc + 1) * 128, 0:L])
                for hc in range(NG):
                    nc.sync.dma_start(out=wk_sb[:, hc, :], in_=wkt_h[hc * 128:(hc + 1) * 128, :])
                nc.sync.dma_start(out=bilir_sb[0:64, :, :, :], in_=bilir_h[:, :, :, :])
                nc.sync.dma_start(out=bilir_sb[64:128, :, :, :], in_=bilir_h[:, :, :, :])
                nc.sync.dma_start(out=mk_b[0][:, 0, :, :], in_=maskt_h[0, 0, :, :, :])
                for hc in range(NG):
                    nc.sync.dma_start(out=wv_sb[:, hc, :], in_=wvt_h[hc * 128:(hc + 1) * 128, :])
                for hc in range(NG):
                    nc.sync.dma_start(out=xt_sb[:, hc, L:TOK], in_=xt_h[hc * 128:(hc + 1) * 128, L:TOK])
                for jc in range(1, NJC):
                    nc.sync.dma_start(out=mk_b[0][:, jc, :, :], in_=maskt_h[0, jc, :, :, :])
                for jc in range(NJC):
                    nc.sync.dma_start(out=mk_b[1][:, jc, :, :], in_=maskt_h[1, jc, :, :, :])

                def emit_qk_unit(b, which, g):
                    # one head-pair group of the q or k projection
                    w_sb, t_sb, bias_sb = (
                        (wq_sb, qt_t[b], bqt_sb) if which == 'q'
                        else (wk_sb, kt_t[b], bkt_sb))
                    ps = scp.tile([128, L], F32, tag="s0", bufs=2,
                                  name=f"pj{which}{b}{g}")
                    for hc in range(NG):
                        nc.tensor.matmul(
                            ps[:, :],
                            lhsT=w_sb[:, hc, g * 128:(g + 1) * 128],
                            rhs=xt_sb[:, hc, b * L:(b + 1) * L],
                            start=(hc == 0), stop=(hc == NG - 1),
                        )
                    nc.scalar.activation(
                        t_sb[0:64, g, :], ps[0:64, :], AF.Identity,
                        bias=bias_sb[0:64, g:g + 1], scale=1.0,
                    )
                    nc.scalar.activation(
                        t_sb[64:128, g, :], ps[64:128, :], AF.Identity,
                        bias=bias_sb[64:128, g:g + 1], scale=1.0,
                    )

                def emit_v_unit(b, jc, ow, osz):
                    ic = b * NJC + jc
                    ps = scp.tile([128, 512], F32, tag="ss", bufs=4,
                                  name=f"vps{ic}{ow}")
                    for hc in range(NG):
                        nc.tensor.matmul(
                            ps[:, 0:osz],
                            lhsT=xt_sb[:, hc, ic * 128:(ic + 1) * 128],
                            rhs=wv_sb[:, hc, ow:ow + osz],
                            start=(hc == 0), stop=(hc == NG - 1),
                        )
                    h0 = ow // 64
                    nh = osz // 64
                    dst = v_t[ic][:, h0 * 65:(h0 + nh) * 65].rearrange(
                        "p (h e) -> p h e", e=65)[:, :, 0:64]
                    # DVE reads the PSUM into the slots; the host-broadcast
                    # bias is added there (Pool cannot read PSUM)
                    nc.vector.tensor_add(
                        dst,
                        ps[:, 0:osz].rearrange("p (h q) -> p h q", q=64),
                        bv_sb[:, ow:ow + osz].rearrange("p (h q) -> p h q", q=64),
                    )

                # minimal upfront (first three head-pairs + early V) so the
                # first attention iterations start ~30us sooner; the rest
                # drips through iteration slots ahead of use.
                emit_qk_unit(0, 'q', 0)
                emit_qk_unit(0, 'k', 0)
                upfront_v = [(0, 0, ow, osz)
                             for ow, osz in ((0, 512), (512, 256))]
                def v_units(b, jc):
                    return [('v', b, jc, ow, osz)
                            for ow, osz in ((0, 512), (512, 256))]

                def qk_units(b, g):
                    return [('qk', b, w, g) for w in ('q', 'k')]

                # ordered so each unit lands before its first consumer
                drip_units = (
                    v_units(0, 1)
                    + qk_units(0, 3) + v_units(0, 2) + v_units(0, 3)
                    + qk_units(0, 4) + qk_units(0, 5) + qk_units(1, 0)
                    + v_units(1, 0) + qk_units(1, 1) + v_units(1, 1)
                    + qk_units(1, 2) + v_units(1, 2) + qk_units(1, 3)
                    + qk_units(1, 4) + v_units(1, 3) + qk_units(1, 5)
                )

                def emit_drip(u):
                    if u[0] == 'qk':
                        emit_qk_unit(u[1], u[2], u[3])
                    else:
                        emit_v_unit(u[1], u[2], u[3], u[4])

                # ---- attention ----
                # Two heads (even/odd of each pair group) are software-
                # pipelined: their iteration bodies interleave so one
                # stream's matmul->STT->combine->exp chain latency hides
                # under the other stream's engine work.
                sgroups = [(0, 2), (2, 4), (4, 5)]

                qs_tiles = {}

                def emit_qs_unit(b, h, gi):
                    """One qs work unit: 1-2 matmuls + one PSUM evacuation.
                    Units are dripped through the schedule so the qs of pair
                    p+2 materializes during pair p's iterations."""
                    p0 = 64 * (h & 1)
                    g = h >> 1
                    qt_head = qt_t[b][p0:p0 + 64, g, :]
                    if (b, h) not in qs_tiles:
                        qs_tiles[(b, h)] = att.tile(
                            [128, NS, L], F16, tag="qs", bufs=6,
                            name=f"qs{b}_{h}")
                    qs_sb = qs_tiles[(b, h)]
                    s0i, s1i = sgroups[gi]
                    for s in range(s0i, s1i):
                        qs_ps = scp.tile([128, L], F32, tag="ss", bufs=4,
                                         name=f"qsps{b}{h}{s}")
                        nc.tensor.matmul(
                            qs_ps[p0:p0 + 64, :],
                            lhsT=bilir_sb[p0:p0 + 64, h, s, :],
                            rhs=qt_head,
                            start=True, stop=True,
                        )
                        if s == 4:  # spread evacuation load off ACT
                            nc.vector.tensor_copy(
                                qs_sb[p0:p0 + 64, s, :], qs_ps[p0:p0 + 64, :])
                        else:
                            nc.scalar.copy(
                                qs_sb[p0:p0 + 64, s, :], qs_ps[p0:p0 + 64, :])

                def emit_scores(b, h, jc, qs_sb):
                    """Phase 1: struct-score matmuls + mask STTs."""
                    p0 = 64 * (h & 1)
                    g = h >> 1
                    kt_j = kt_t[b][p0:p0 + 64, g, jc * 128:(jc + 1) * 128]
                    u_d = att.tile([128, 3, L], F16, tag="ud", bufs=4,
                                   name=f"ud{b}{h}{jc}")
                    u_p = att.tile([128, 2, L], F16, tag="up", bufs=4,
                                   name=f"up{b}{h}{jc}")
                    ssc = att.tile([128, 2, L], F16, tag="ssc", bufs=4,
                                   name=f"ssc{b}{h}{jc}")
                    nd = 0
                    npl = 0
                    uslot = {}
                    for s in range(NS):
                        ss = scp.tile([128, L], F32, tag="ss", bufs=4,
                                      name=f"ss{b}{h}{jc}{s}")
                        nc.tensor.matmul(
                            ss[:, :],
                            lhsT=kt_j,
                            rhs=qs_sb[p0:p0 + 64, s, :],
                            start=True, stop=True,
                        )
                        ab = absb_sb[:, s * H + h:s * H + h + 1]
                        mk = mk_b[b][:, jc, s, :]
                        if STT_ENG[s] == 'd':
                            dst = u_d[:, nd, :]
                            uslot[s] = dst
                            nd += 1
                            nc.vector.scalar_tensor_tensor(
                                dst, ss[:, :], ab, mk, OP.add, OP.mult)
                        else:
                            # ACT evacuates (ss + ab) to SBUF; Pool masks it
                            nc.scalar.activation(
                                ssc[:, npl, :], ss[:, :], AF.Identity,
                                bias=ab, scale=1.0)
                            dst = u_p[:, npl, :]
                            uslot[s] = dst
                            nc.gpsimd.tensor_mul(dst, ssc[:, npl, :], mk)
                            npl += 1
                    return uslot

                def emit_combine(b, h, jc, uslot, ctx_ps):
                    """Phase 2: S0 matmul, combines, exp, context matmul."""
                    p0 = 64 * (h & 1)
                    g = h >> 1
                    qt_head = qt_t[b][p0:p0 + 64, g, :]
                    kt_j = kt_t[b][p0:p0 + 64, g, jc * 128:(jc + 1) * 128]
                    # structs 2-4 summed on DVE (f16 2x adds)
                    ta = att.tile([128, L], F16, tag="ta", bufs=3,
                                  name=f"ta{b}{h}{jc}")
                    nc.vector.tensor_add(ta[:, :], uslot[TREE[0]],
                                         uslot[TREE[1]])
                    tb = att.tile([128, L], F16, tag="tb", bufs=3,
                                  name=f"tb{b}{h}{jc}")
                    nc.vector.tensor_add(tb[:, :], ta[:, :], uslot[TREE[2]])
                    s0 = scp.tile([128, L], F32, tag="s0", bufs=2,
                                  name=f"s0_{b}_{h}_{jc}")
                    nc.tensor.matmul(s0[:, :], lhsT=kt_j, rhs=qt_head,
                                     start=True, stop=False,
                                     skip_group_check=True)
                    # structs 0-2 + the pair sum: PE identities into s0
                    adds = [uslot[s] for s in IDS] + [tb[:, :]]
                    for i, rhs in enumerate(adds):
                        nc.tensor.matmul(s0[:, :], lhsT=ident[:, :],
                                         rhs=rhs,
                                         start=False,
                                         stop=(i == len(adds) - 1),
                                         skip_group_check=True)
                    pr = att.tile([128, L], F16, tag="pr", bufs=4,
                                  name=f"pr{b}{h}{jc}")
                    nc.scalar.activation(
                        pr[:, :], s0[:, :], AF.Exp,
                        bias=amt_sb[:, b * NJC + jc:b * NJC + jc + 1],
                        scale=1.0,
                    )
                    nc.tensor.matmul(
                        ctx_ps[0:65, :],
                        lhsT=v_t[b * NJC + jc][:, h * 65:(h + 1) * 65],
                        rhs=pr[:, :],
                        start=(jc == 0), stop=(jc == NJC - 1),
                        skip_group_check=True,
                    )

                pairs = [(b, g) for b in range(NB) for g in range(NG)]

                def qs_units(pi):
                    if pi >= len(pairs):
                        return []
                    b_, g_ = pairs[pi]
                    return [(b_, 2 * g_ + hi, gi)
                            for hi in range(2) for gi in range(3)]

                # cold-start sequenced along the first pair's critical path:
                # pair 0's qs right after its projections, then the next two
                # head-pair projections and pair 1's qs; pair p+2's qs drips
                # through pair p's iteration slots.
                for unit in qs_units(0):
                    emit_qs_unit(*unit)
                emit_qk_unit(0, 'q', 1)
                emit_qk_unit(0, 'k', 1)
                # V(b0) after the qs cold-start so a wv DMA wait cannot
                # block the first score matmuls
                for u in upfront_v:
                    emit_v_unit(*u)
                for unit in qs_units(1):
                    emit_qs_unit(*unit)
                emit_qk_unit(0, 'q', 2)
                emit_qk_unit(0, 'k', 2)
                for pi, (b, g) in enumerate(pairs):
                    heads = (2 * g, 2 * g + 1)
                    qs_pair = [qs_tiles[(b, h)] for h in heads]
                    drip = list(qs_units(pi + 2))
                    ctx_pair = [ctxp.tile([65, L], F32, tag="ctx",
                                          name=f"ctx{b}_{h}")
                                for h in heads]
                    for jc in range(NJC):
                        us = []
                        for hi, h in enumerate(heads):
                            us.append(emit_scores(b, h, jc, qs_pair[hi]))
                            if drip:
                                emit_qs_unit(*drip.pop(0))
                        for _ in range(2 if pi < 3 else 1):
                            if drip_units:
                                emit_drip(drip_units.pop(0))
                        for hi, h in enumerate(heads):
                            emit_combine(b, h, jc, us[hi], ctx_pair[hi])
                    # raw ctx^T + denominator row to DRAM via an SBUF
                    # bounce; the host transposes and normalizes.
                    for hi, h in enumerate(heads):
                        ct = att.tile([65, L], F16, tag="ct", bufs=4,
                                      name=f"ct{b}_{h}")
                        nc.scalar.copy(ct[:, :], ctx_pair[hi][0:65, :])
                        nc.sync.dma_start(out=out_h[b, h, :, :], in_=ct[:, :])
    return nc


_NC = None


def _get_nc():
    global _NC
    if _NC is None:
        _NC = _build_nc()
    return _NC


def _prep_inputs(inputs):
    hs = np.asarray(inputs["hidden_states"], np.float32)
    am = np.asarray(inputs["attention_mask"], np.float32).reshape(B, L)
    sm = np.asarray(inputs["structure_mask"], np.float32)
    Wq = np.asarray(inputs["Wq"], np.float32)
    Wk = np.asarray(inputs["Wk"], np.float32)
    Wv = np.asarray(inputs["Wv"], np.float32)
    bq = np.asarray(inputs["bq"], np.float32)
    bk = np.asarray(inputs["bk"], np.float32)
    bv = np.asarray(inputs["bv"], np.float32)
    bili = np.asarray(inputs["bili"], np.float32)
    absb = np.asarray(inputs["abs_bias"], np.float32)

    sc = np.float32(1.0 / np.sqrt(D))  # folded into the whole q side
    shared = {
        "wqt": np.ascontiguousarray(Wq.T * sc).astype(np.float16),
        "wkt": np.ascontiguousarray(Wk.T).astype(np.float16),
        "wvt": np.ascontiguousarray(Wv.T).astype(np.float16),
        "bqt": np.ascontiguousarray((bq * sc).reshape(NG, 128).T),
        "bkt": np.ascontiguousarray(bk.reshape(NG, 128).T),
        "bv": bv,
        # [d, h, s, p] so lhsT slice [:, h, s:s+2, :] pairs two structs
        "bilir": np.ascontiguousarray(
            bili.transpose(2, 1, 0, 3)).astype(np.float16),
        "absb": np.ascontiguousarray(absb.reshape(NS * H) * sc),
    }
    in_maps = []
    for c in range(NCORES):
        b0 = c * NB
        x = hs[b0:b0 + NB].reshape(TOK, HID)
        amc = am[b0:b0 + NB]  # [NB, L]
        # -10: constant logit shift (softmax-invariant) keeping exp() and the
        # row sums inside fp16 range without a max-reduction pass.
        amt = np.ascontiguousarray(
            amc.reshape(NB, NJC, 128).transpose(2, 0, 1)).reshape(128, NB * NJC) - 10.0
        mk = sm[:, b0:b0 + NB, 0]  # [NS, NB, L(i), L(j)]
        mkt = np.ascontiguousarray(mk.transpose(1, 3, 0, 2))  # [NB, j, NS, i]
        mkt = mkt.reshape(NB, NJC, 128, NS, L).astype(np.float16)
        in_maps.append(dict(
            xt=np.ascontiguousarray(x.T).astype(np.float16), amt=amt, maskt=mkt, **shared))
    return in_maps


def kernel(**inputs):
    global LAST_RESULT
    nc = _get_nc()
    in_maps = _prep_inputs(inputs)
    import os
    trace = bool(os.environ.get("BASS_TRACE"))
    try:
        LAST_RESULT = run_bass_kernel_spmd(
            nc, in_maps, core_ids=list(range(NCORES)), trace=trace)
    except ModuleNotFoundError:
        # axon NTFF profile hook unavailable in this environment
        LAST_RESULT = run_bass_kernel_spmd(
            nc, in_maps, core_ids=list(range(NCORES)), trace=False)
    outs = np.stack([r["out"] for r in LAST_RESULT.results]).astype(np.float32)
    den = outs[:, :, :, 64:65, :]  # [8, NB, H, 1, L]
    vals = outs[:, :, :, 0:64, :] / den
    # [c, b, h, d, i] -> [c, b, i, h, d] -> [B, L, HID]
    return np.ascontiguousarray(vals.transpose(0, 1, 4, 2, 3)).reshape(B, L, HID)


# revision 85
# speedup vs baseline: 1.0161x; 1.0033x over previous
"""Trainium2 Bass kernel for masked-biaffine BERT self-attention.

Data-parallel over batch (16 batches / 8 cores = 2 per core). Scores are
computed transposed (S[j,i], keys on partitions) so the additive
attention_mask is a per-partition exp bias and softmax normalization comes
free from a ones-column in the V matmul.

Per (b,h,jc) iteration:
  - q/k projections pair two heads per matmul ([128,512] PSUM tiles);
  - the five mask applications (ss+ab)*m split across engines: structs 0-2
    as DVE STTs straight from PSUM, structs 3-4 via an ACT bias-copy to
    SBUF (+ab rides the copy) followed by a Pool tensor_mul;
  - structs 0-1 and the DVE-summed 2-4 join the S0 PSUM bank through three
    PE identity matmuls, so exp reads PSUM directly;
  - context accumulates in PSUM with a ones-column denominator row, is
    copied once to SBUF f16 and DMA'd out; the transpose and
    1/denominator normalization happen on the host.
Work for batch 1 and later head-pairs drips through earlier iteration
slots (the engines are in-order, so emission order is the schedule).
1/sqrt(D) is folded into Wq/bq/abs_bias on the host; exp runs with scale=1.
"""

import sys

if "/opt/trn_rl_repo" not in sys.path:
    sys.path.insert(0, "/opt/trn_rl_repo")

import json

import numpy as np

import concourse.bass as bass
import concourse.mybir as mybir
import concourse.tile as tile
from concourse.masks import make_identity
from concourse.bass_utils import run_bass_kernel_spmd

# ---- BIR post-pass: this walrus build allows only one sync_info.on_wait ----
# entry per instruction; hoist extras onto inserted NoOps on the same engine.
_MAXW = 1
_split_ctr = [0]


def _split_waits_json(j):
    nsplit = 0
    for fn in j.get("functions", []):
        for blk in fn.get("blocks", []):
            out = []
            for body in blk.get("instructions", []):
                si = body.get("sync_info") or {}
                ow = si.get("on_wait") or []
                if len(ow) > _MAXW:
                    extra = ow[:-_MAXW]
                    si["on_wait"] = ow[-_MAXW:]
                    while extra:
                        grp, extra = extra[:_MAXW], extra[_MAXW:]
                        _split_ctr[0] += 1
                        out.append({
                            "debug": body.get("debug", 0),
                            "engine": body["engine"],
                            "ins": [],
                            "name": f"I-waitsplit-{_split_ctr[0]}",
                            "opcode": "NoOp",
                            "outs": [],
                            "sync_info": {"on_update": [], "on_wait": grp},
                        })
                    nsplit += 1
                out.append(body)
            blk["instructions"] = out
    return nsplit


def _install_birfix():
    import concourse.bass_utils as bu
    import concourse.bass2jax as b2j

    if getattr(bu, "_waitsplit_installed", False):
        return
    orig = bu.compile_bir_kernel

    def patched(bir_json, tmpdir, neff_name="file.neff"):
        j = json.loads(bir_json)
        _split_waits_json(j)
        return orig(json.dumps(j).encode(), tmpdir, neff_name)

    bu.compile_bir_kernel = patched
    b2j.compile_bir_kernel = patched
    bu._waitsplit_installed = True


_install_birfix()

B, L, HID, H, D = 16, 512, 768, 12, 64
NS = 5
NCORES = 8
NB = B // NCORES          # batches per core
TOK = NB * L              # tokens per core
NJC = L // 128            # j-chunks per (b,h)
NG = HID // 128           # head-pair groups (6)
F32 = mybir.dt.float32
F16 = mybir.dt.float16
AF = mybir.ActivationFunctionType
OP = mybir.AluOpType

# struct -> mask-apply path. GPSIMD cannot read PSUM (and only supports
# plain TensorTensor ops), so:
#   'd' = DVE STT straight from PSUM;
#   'a' = ACT copy PSUM->SBUF f16 (the +ab bias rides the copy), then the
#         mask-multiply runs on Pool as tensor_mul from SBUF.
STT_ENG = ['d', 'd', 'd', 'a', 'a']
IDS = [0, 1]        # structs identity-accumulated on PE into the S0 bank
TREE = [2, 3, 4]    # summed by two DVE adds; joins s0 via a third identity

LAST_RESULT = None  # BassKernelResults of the most recent run (for test.py)


def _build_nc():
    nc = bass.Bass()

    # ---- DRAM I/O (per core) ----
    xt_h = nc.dram_tensor("xt", [HID, TOK], F16, kind="ExternalInput")
    wqt_h = nc.dram_tensor("wqt", [HID, HID], F16, kind="ExternalInput")
    wkt_h = nc.dram_tensor("wkt", [HID, HID], F16, kind="ExternalInput")
    wvt_h = nc.dram_tensor("wvt", [HID, HID], F16, kind="ExternalInput")
    bqt_h = nc.dram_tensor("bqt", [128, NG], F32, kind="ExternalInput")
    bkt_h = nc.dram_tensor("bkt", [128, NG], F32, kind="ExternalInput")
    bv_h = nc.dram_tensor("bv", [HID], F32, kind="ExternalInput")
    bilir_h = nc.dram_tensor("bilir", [D, H, NS, D], F16, kind="ExternalInput")
    absb_h = nc.dram_tensor("absb", [NS * H], F32, kind="ExternalInput")
    amt_h = nc.dram_tensor("amt", [128, NB * NJC], F32, kind="ExternalInput")
    maskt_h = nc.dram_tensor("maskt", [NB, NJC, 128, NS, L], F16, kind="ExternalInput")
    # [b, h, 65, i]: rows 0..63 = unnormalized ctx^T, row 64 = softmax denom
    out_h = nc.dram_tensor("out", [NB, H, 65, L], F16, kind="ExternalOutput")

    with tile.TileContext(nc) as tc:
        with tc.tile_pool(name="pers", bufs=1) as pers:
            # persistent SBUF tensors
            # paired q/k: group g holds heads (2g, 2g+1) on partition halves
            qt_t = [pers.tile([128, NG, L], F16, tag=f"qt{b}", name=f"qt{b}")
                    for b in range(NB)]
            kt_t = [pers.tile([128, NG, L], F16, tag=f"kt{b}", name=f"kt{b}")
                    for b in range(NB)]
            v_t = [pers.tile([128, H * 65], F16, tag=f"v{ic}", name=f"v{ic}")
                   for ic in range(NB * NJC)]
            # both partition halves hold the same [d, h, s, p] data so lhsT
            # can start at partition 0 or 64 to match the head's parity
            bilir_sb = pers.tile([128, H, NS, D], F16, tag="bilir")
            absb_sb = pers.tile([128, NS * H], F32, tag="absb")
            amt_sb = pers.tile([128, NB * NJC], F32, tag="amt")
            bqt_sb = pers.tile([128, NG], F32, tag="bqt")
            bkt_sb = pers.tile([128, NG], F32, tag="bkt")
            bv_sb = pers.tile([128, HID], F32, tag="bv")
            ident = pers.tile([128, 128], F16, tag="ident")

            make_identity(nc, ident[:, :])
            nc.sync.dma_start(out=amt_sb[:, :], in_=amt_h[:, :])
            nc.sync.dma_start(out=bqt_sb[:, :], in_=bqt_h[:, :])
            nc.sync.dma_start(out=bkt_sb[:, :], in_=bkt_h[:, :])
            ab_ap = absb_h[:]
            nc.gpsimd.dma_start(
                out=absb_sb[:, :],
                in_=bass.AP(tensor=ab_ap.tensor, offset=ab_ap.offset,
                            ap=[[0, 128], [1, NS * H]]),
            )
            bv_ap = bv_h[:]
            nc.gpsimd.dma_start(
                out=bv_sb[:, :],
                in_=bass.AP(tensor=bv_ap.tensor, offset=bv_ap.offset,
                            ap=[[0, 128], [1, HID]]),
            )
            # ones columns of v_ext: preset whole tile to 1.0; projection
            # evacuations overwrite the 64 value columns of each head slot.
            for ic in range(NB * NJC):
                nc.vector.memset(v_t[ic][:, :], 1.0)

            with (
                tc.tile_pool(name="stageb", bufs=1) as stb,
                tc.tile_pool(name="mpool", bufs=2) as mpool,
                tc.tile_pool(name="att", bufs=2) as att,
                tc.tile_pool(name="sc_ps", bufs=1, space="PSUM") as scp,
                tc.tile_pool(name="ctx_ps", bufs=2, space="PSUM") as ctxp,
            ):
                xt_sb = stb.tile([128, NG, TOK], F16, tag="xt")
                wq_sb = stb.tile([128, NG, HID], F16, tag="wq")
                wk_sb = stb.tile([128, NG, HID], F16, tag="wk")
                wv_sb = stb.tile([128, NG, HID], F16, tag="wv")
                # masks for batch b, loaded once, reused by all 12 heads
                mk_b = [mpool.tile([128, NJC, NS, L], F16, tag="mask",
                                   name=f"mask{b}") for b in range(NB)]

                # DMA queue ordered along the first iteration's critical
                # path: q's operands, then the first mask chunk, then the
                # rest interleaved.
                # batch-0's x half first: the first projection matmuls gate
                # on (wq chunk, xt-b0 chunk) pairs
                # DMA queue ordered along the first iterations' critical
                # path: q then k operands, bilir (for qs), first mask
                # chunk, then the rest.
                for hc in range(NG):
                    nc.sync.dma_start(out=wq_sb[:, hc, :], in_=wqt_h[hc * 128:(hc + 1) * 128, :])
                    nc.sync.dma_start(out=xt_sb[:, hc, 0:L], in_=xt_h[hc * 128:(hc + 1) * 128, 0:L])
                for hc in range(NG):
                    nc.sync.dma_start(out=wk_sb[:, hc, :], in_=wkt_h[hc * 128:(hc + 1) * 128, :])
                nc.sync.dma_start(out=bilir_sb[0:64, :, :, :], in_=bilir_h[:, :, :, :])
                nc.sync.dma_start(out=bilir_sb[64:128, :, :, :], in_=bilir_h[:, :, :, :])
                nc.sync.dma_start(out=mk_b[0][:, 0, :, :], in_=maskt_h[0, 0, :, :, :])
                for hc in range(NG):
                    nc.sync.dma_start(out=wv_sb[:, hc, :], in_=wvt_h[hc * 128:(hc + 1) * 128, :])
                for hc in range(NG):
                    nc.sync.dma_start(out=xt_sb[:, hc, L:TOK], in_=xt_h[hc * 128:(hc + 1) * 128, L:TOK])
                for jc in range(1, NJC):
                    nc.sync.dma_start(out=mk_b[0][:, jc, :, :], in_=maskt_h[0, jc, :, :, :])
                for jc in range(NJC):
                    nc.sync.dma_start(out=mk_b[1][:, jc, :, :], in_=maskt_h[1, jc, :, :, :])

                def emit_qk_unit(b, which, g):
                    # one head-pair group of the q or k projection
                    w_sb, t_sb, bias_sb = (
                        (wq_sb, qt_t[b], bqt_sb) if which == 'q'
                        else (wk_sb, kt_t[b], bkt_sb))
                    ps = scp.tile([128, L], F32, tag="s0", bufs=2,
                                  name=f"pj{which}{b}{g}")
                    for hc in range(NG):
                        nc.tensor.matmul(
                            ps[:, :],
                            lhsT=w_sb[:, hc, g * 128:(g + 1) * 128],
                            rhs=xt_sb[:, hc, b * L:(b + 1) * L],
                            start=(hc == 0), stop=(hc == NG - 1),
                        )
                    nc.scalar.activation(
                        t_sb[0:64, g, :], ps[0:64, :], AF.Identity,
                        bias=bias_sb[0:64, g:g + 1], scale=1.0,
                    )
                    nc.scalar.activation(
                        t_sb[64:128, g, :], ps[64:128, :], AF.Identity,
                        bias=bias_sb[64:128, g:g + 1], scale=1.0,
                    )

                def emit_v_unit(b, jc, ow, osz):
                    ic = b * NJC + jc
                    ps = scp.tile([128, 512], F32, tag="ss", bufs=4,
                                  name=f"vps{ic}{ow}")
                    for hc in range(NG):
                        nc.tensor.matmul(
                            ps[:, 0:osz],
                            lhsT=xt_sb[:, hc, ic * 128:(ic + 1) * 128],
                            rhs=wv_sb[:, hc, ow:ow + osz],
                            start=(hc == 0), stop=(hc == NG - 1),
                        )
                    h0 = ow // 64
                    nh = osz // 64
                    dst = v_t[ic][:, h0 * 65:(h0 + nh) * 65].rearrange(
                        "p (h e) -> p h e", e=65)[:, :, 0:64]
                    # DVE reads the PSUM into the slots; the host-broadcast
                    # bias is added there (Pool cannot read PSUM)
                    nc.vector.tensor_add(
                        dst,
                        ps[:, 0:osz].rearrange("p (h q) -> p h q", q=64),
                        bv_sb[:, ow:ow + osz].rearrange("p (h q) -> p h q", q=64),
                    )

                # minimal upfront (first three head-pairs + early V) so the
                # first attention iterations start ~30us sooner; the rest
                # drips through iteration slots ahead of use.
                emit_qk_unit(0, 'q', 0)
                emit_qk_unit(0, 'k', 0)
                upfront_v = [(0, 0, ow, osz)
                             for ow, osz in ((0, 512), (512, 256))]
                def v_units(b, jc):
                    return [('v', b, jc, ow, osz)
                            for ow, osz in ((0, 512), (512, 256))]

                def qk_units(b, g):
                    return [('qk', b, w, g) for w in ('q', 'k')]

                # ordered so each unit lands before its first consumer
                drip_units = (
                    v_units(0, 1)
                    + qk_units(0, 3) + v_units(0, 2) + v_units(0, 3)
                    + qk_units(0, 4) + qk_units(0, 5) + qk_units(1, 0)
                    + v_units(1, 0) + qk_units(1, 1) + v_units(1, 1)
                    + qk_units(1, 2) + v_units(1, 2) + qk_units(1, 3)
                    + qk_units(1, 4) + v_units(1, 3) + qk_units(1, 5)
                )

                def emit_drip(u):
                    if u[0] == 'qk':
                        emit_qk_unit(u[1], u[2], u[3])
                    else:
                        emit_v_unit(u[1], u[2], u[3], u[4])

                # ---- attention ----
                # Two heads (even/odd of each pair group) are software-
                # pipelined: their iteration bodies interleave so one
                # stream's matmul->STT->combine->exp chain latency hides
                # under the other stream's engine work.
                sgroups = [(0, 2), (2, 4), (4, 5)]

                qs_tiles = {}

                def emit_qs_unit(b, h, gi):
                    """One qs work unit: 1-2 matmuls + one PSUM evacuation.
                    Units are dripped through the schedule so the qs of pair
                    p+2 materializes during pair p's iterations."""
                    p0 = 64 * (h & 1)
                    g = h >> 1
                    qt_head = qt_t[b][p0:p0 + 64, g, :]
                    if (b, h) not in qs_tiles:
                        qs_tiles[(b, h)] = att.tile(
                            [128, NS, L], F16, tag="qs", bufs=6,
                            name=f"qs{b}_{h}")
                    qs_sb = qs_tiles[(b, h)]
                    s0i, s1i = sgroups[gi]
                    for s in range(s0i, s1i):
                        qs_ps = scp.tile([128, L], F32, tag="ss", bufs=4,
                                         name=f"qsps{b}{h}{s}")
                        nc.tensor.matmul(
                            qs_ps[p0:p0 + 64, :],
                            lhsT=bilir_sb[p0:p0 + 64, h, s, :],
                            rhs=qt_head,
                            start=True, stop=True,
                        )
                        if s == 4:  # spread evacuation load off ACT
                            nc.vector.tensor_copy(
                                qs_sb[p0:p0 + 64, s, :], qs_ps[p0:p0 + 64, :])
                        else:
                            nc.scalar.copy(
                                qs_sb[p0:p0 + 64, s, :], qs_ps[p0:p0 + 64, :])

                def emit_scores(b, h, jc, qs_sb):
                    """Phase 1: struct-score matmuls + mask STTs."""
                    p0 = 64 * (h & 1)
                    g = h >> 1
                    kt_j = kt_t[b][p0:p0 + 64, g, jc * 128:(jc + 1) * 128]
                    u_d = att.tile([128, 3, L], F16, tag="ud", bufs=4,
                                   name=f"ud{b}{h}{jc}")
                    u_p = att.tile([128, 2, L], F16, tag="up", bufs=4,
                                   name=f"up{b}{h}{jc}")
                    ssc = att.tile([128, 2, L], F16, tag="ssc", bufs=4,
                                   name=f"ssc{b}{h}{jc}")
                    nd = 0
                    npl = 0
                    uslot = {}
                    for s in range(NS):
                        ss = scp.tile([128, L], F32, tag="ss", bufs=4,
                                      name=f"ss{b}{h}{jc}{s}")
                        nc.tensor.matmul(
                            ss[:, :],
                            lhsT=kt_j,
                            rhs=qs_sb[p0:p0 + 64, s, :],
                            start=True, stop=True,
                        )
                        ab = absb_sb[:, s * H + h:s * H + h + 1]
                        mk = mk_b[b][:, jc, s, :]
                        if STT_ENG[s] == 'd':
                            dst = u_d[:, nd, :]
                            uslot[s] = dst
                            nd += 1
                            nc.vector.scalar_tensor_tensor(
                                dst, ss[:, :], ab, mk, OP.add, OP.mult)
                        else:
                            # ACT evacuates (ss + ab) to SBUF; Pool masks it
                            nc.scalar.activation(
                                ssc[:, npl, :], ss[:, :], AF.Identity,
                                bias=ab, scale=1.0)
                            dst = u_p[:, npl, :]
                            uslot[s] = dst
                            nc.gpsimd.tensor_mul(dst, ssc[:, npl, :], mk)
                            npl += 1
                    return uslot

                def combine_tree(b, h, jc, uslot):
                    """Structs 2-4 summed on DVE (f16 2x adds)."""
                    ta = att.tile([128, L], F16, tag="ta", bufs=3,
                                  name=f"ta{b}{h}{jc}")
                    nc.vector.tensor_add(ta[:, :], uslot[TREE[0]],
                                         uslot[TREE[1]])
                    tb = att.tile([128, L], F16, tag="tb", bufs=3,
                                  name=f"tb{b}{h}{jc}")
                    nc.vector.tensor_add(tb[:, :], ta[:, :], uslot[TREE[2]])
                    return tb

                def combine_pe(b, h, jc, uslot, tb, ctx_ps):
                    """S0 matmul, identity accumulates, exp, context."""
                    p0 = 64 * (h & 1)
                    g = h >> 1
                    qt_head = qt_t[b][p0:p0 + 64, g, :]
                    kt_j = kt_t[b][p0:p0 + 64, g, jc * 128:(jc + 1) * 128]
                    s0 = scp.tile([128, L], F32, tag="s0", bufs=2,
                                  name=f"s0_{b}_{h}_{jc}")
                    nc.tensor.matmul(s0[:, :], lhsT=kt_j, rhs=qt_head,
                                     start=True, stop=False,
                                     skip_group_check=True)
                    # structs 0-1 + the tree sum: PE identities into s0
                    adds = [uslot[s] for s in IDS] + [tb[:, :]]
                    for i, rhs in enumerate(adds):
                        nc.tensor.matmul(s0[:, :], lhsT=ident[:, :],
                                         rhs=rhs,
                                         start=False,
                                         stop=(i == len(adds) - 1),
                                         skip_group_check=True)
                    pr = att.tile([128, L], F16, tag="pr", bufs=4,
                                  name=f"pr{b}{h}{jc}")
                    nc.scalar.activation(
                        pr[:, :], s0[:, :], AF.Exp,
                        bias=amt_sb[:, b * NJC + jc:b * NJC + jc + 1],
                        scale=1.0,
                    )
                    nc.tensor.matmul(
                        ctx_ps[0:65, :],
                        lhsT=v_t[b * NJC + jc][:, h * 65:(h + 1) * 65],
                        rhs=pr[:, :],
                        start=(jc == 0), stop=(jc == NJC - 1),
                        skip_group_check=True,
                    )

                pairs = [(b, g) for b in range(NB) for g in range(NG)]

                def qs_units(pi):
                    if pi >= len(pairs):
                        return []
                    b_, g_ = pairs[pi]
                    return [(b_, 2 * g_ + hi, gi)
                            for hi in range(2) for gi in range(3)]

                # cold-start sequenced along the first pair's critical path:
                # pair 0's qs right after its projections, then the next two
                # head-pair projections and pair 1's qs; pair p+2's qs drips
                # through pair p's iteration slots.
                for unit in qs_units(0):
                    emit_qs_unit(*unit)
                emit_qk_unit(0, 'q', 1)
                emit_qk_unit(0, 'k', 1)
                # V(b0) after the qs cold-start so a wv DMA wait cannot
                # block the first score matmuls
                for u in upfront_v:
                    emit_v_unit(*u)
                for unit in qs_units(1):
                    emit_qs_unit(*unit)
                emit_qk_unit(0, 'q', 2)
                emit_qk_unit(0, 'k', 2)
                for pi, (b, g) in enumerate(pairs):
                    heads = (2 * g, 2 * g + 1)
                    qs_pair = [qs_tiles[(b, h)] for h in heads]
                    drip = list(qs_units(pi + 2))
                    ctx_pair = [ctxp.tile([65, L], F32, tag="ctx",
                                          name=f"ctx{b}_{h}")
                                for h in heads]
                    for jc in range(NJC):
                        us = []
                        for hi, h in enumerate(heads):
                            us.append(emit_scores(b, h, jc, qs_pair[hi]))
                        for _ in range(2):
                            if drip:
                                emit_qs_unit(*drip.pop(0))
                        for _ in range(2 if pi < 3 else 1):
                            if drip_units:
                                emit_drip(drip_units.pop(0))
                        # both heads' DVE trees first, then their PE/ACT
                        # chains, so each engine sees back-to-back work
                        tbs = [combine_tree(b, h, jc, us[hi])
                               for hi, h in enumerate(heads)]
                        for hi, h in enumerate(heads):
                            combine_pe(b, h, jc, us[hi], tbs[hi],
                                       ctx_pair[hi])
                    # raw ctx^T + denominator row to DRAM via an SBUF
                    # bounce; the host transposes and normalizes.
                    for hi, h in enumerate(heads):
                        ct = att.tile([65, L], F16, tag="ct", bufs=4,
                                      name=f"ct{b}_{h}")
                        nc.scalar.copy(ct[:, :], ctx_pair[hi][0:65, :])
                        nc.sync.dma_start(out=out_h[b, h, :, :], in_=ct[:, :])
    return nc


_NC = None


def _get_nc():
    global _NC
    if _NC is None:
        _NC = _build_nc()
    return _NC


def _prep_inputs(inputs):
    hs = np.asarray(inputs["hidden_states"], np.float32)
    am = np.asarray(inputs["attention_mask"], np.float32).reshape(B, L)
    sm = np.asarray(inputs["structure_mask"], np.float32)
    Wq = np.asarray(inputs["Wq"], np.float32)
    Wk = np.asarray(inputs["Wk"], np.float32)
    Wv = np.asarray(inputs["Wv"], np.float32)
    bq = np.asarray(inputs["bq"], np.float32)
    bk = np.asarray(inputs["bk"], np.float32)
    bv = np.asarray(inputs["bv"], np.float32)
    bili = np.asarray(inputs["bili"], np.float32)
    absb = np.asarray(inputs["abs_bias"], np.float32)

    sc = np.float32(1.0 / np.sqrt(D))  # folded into the whole q side
    shared = {
        "wqt": np.ascontiguousarray(Wq.T * sc).astype(np.float16),
        "wkt": np.ascontiguousarray(Wk.T).astype(np.float16),
        "wvt": np.ascontiguousarray(Wv.T).astype(np.float16),
        "bqt": np.ascontiguousarray((bq * sc).reshape(NG, 128).T),
        "bkt": np.ascontiguousarray(bk.reshape(NG, 128).T),
        "bv": bv,
        # [d, h, s, p] so lhsT slice [:, h, s:s+2, :] pairs two structs
        "bilir": np.ascontiguousarray(
            bili.transpose(2, 1, 0, 3)).astype(np.float16),
        "absb": np.ascontiguousarray(absb.reshape(NS * H) * sc),
    }
    in_maps = []
    for c in range(NCORES):
        b0 = c * NB
        x = hs[b0:b0 + NB].reshape(TOK, HID)
        amc = am[b0:b0 + NB]  # [NB, L]
        # -10: constant logit shift (softmax-invariant) keeping exp() and the
        # row sums inside fp16 range without a max-reduction pass.
        amt = np.ascontiguousarray(
            amc.reshape(NB, NJC, 128).transpose(2, 0, 1)).reshape(128, NB * NJC) - 10.0
        mk = sm[:, b0:b0 + NB, 0]  # [NS, NB, L(i), L(j)]
        mkt = np.ascontiguousarray(mk.transpose(1, 3, 0, 2))  # [NB, j, NS, i]
        mkt = mkt.reshape(NB, NJC, 128, NS, L).astype(np.float16)
        in_maps.append(dict(
            xt=np.ascontiguousarray(x.T).astype(np.float16), amt=amt, maskt=mkt, **shared))
    return in_maps


def kernel(**inputs):
    global LAST_RESULT
    nc = _get_nc()
    in_maps = _prep_inputs(inputs)
    import os
    trace = bool(os.environ.get("BASS_TRACE"))
    try:
        LAST_RESULT = run_bass_kernel_spmd(
            nc, in_maps, core_ids=list(range(NCORES)), trace=trace)
    except ModuleNotFoundError:
        # axon NTFF profile hook unavailable in this environment
        LAST_RESULT = run_bass_kernel_spmd(
            nc, in_maps, core_ids=list(range(NCORES)), trace=False)
    outs = np.stack([r["out"] for r in LAST_RESULT.results]).astype(np.float32)
    den = outs[:, :, :, 64:65, :]  # [8, NB, H, 1, L]
    vals = outs[:, :, :, 0:64, :] / den
    # [c, b, h, d, i] -> [c, b, i, h, d] -> [B, L, HID]
    return np.ascontiguousarray(vals.transpose(0, 1, 4, 2, 3)).reshape(B, L, HID)


# revision 88
# speedup vs baseline: 1.0486x; 1.0319x over previous
"""Trainium2 Bass kernel for masked-biaffine BERT self-attention.

Data-parallel over batch (16 batches / 8 cores = 2 per core). Scores are
computed transposed (S[j,i], keys on partitions) so the additive
attention_mask is a per-partition exp bias and softmax normalization comes
free from a ones-column in the V matmul.

Per (b,h,jc) iteration:
  - q/k projections pair two heads per matmul ([128,512] PSUM tiles);
  - the five mask applications (ss+ab)*m split across engines: structs 0-2
    as DVE STTs straight from PSUM, structs 3-4 via an ACT bias-copy to
    SBUF (+ab rides the copy) followed by a Pool tensor_mul;
  - structs 0-1 and the DVE-summed 2-4 join the S0 PSUM bank through three
    PE identity matmuls, so exp reads PSUM directly;
  - context accumulates in PSUM with a ones-column denominator row, is
    copied once to SBUF f16 and DMA'd out; the transpose and
    1/denominator normalization happen on the host.
Work for batch 1 and later head-pairs drips through earlier iteration
slots (the engines are in-order, so emission order is the schedule).
1/sqrt(D) is folded into Wq/bq/abs_bias on the host; exp runs with scale=1.
"""

import sys

if "/opt/trn_rl_repo" not in sys.path:
    sys.path.insert(0, "/opt/trn_rl_repo")

import json

import numpy as np

import concourse.bass as bass
import concourse.mybir as mybir
import concourse.tile as tile
from concourse.masks import make_identity
from concourse.bass_utils import run_bass_kernel_spmd

# ---- BIR post-pass: this walrus build allows only one sync_info.on_wait ----
# entry per instruction; hoist extras onto inserted NoOps on the same engine.
_MAXW = 1
_split_ctr = [0]


def _split_waits_json(j):
    nsplit = 0
    for fn in j.get("functions", []):
        for blk in fn.get("blocks", []):
            out = []
            for body in blk.get("instructions", []):
                si = body.get("sync_info") or {}
                ow = si.get("on_wait") or []
                if len(ow) > _MAXW:
                    extra = ow[:-_MAXW]
                    si["on_wait"] = ow[-_MAXW:]
                    while extra:
                        grp, extra = extra[:_MAXW], extra[_MAXW:]
                        _split_ctr[0] += 1
                        out.append({
                            "debug": body.get("debug", 0),
                            "engine": body["engine"],
                            "ins": [],
                            "name": f"I-waitsplit-{_split_ctr[0]}",
                            "opcode": "NoOp",
                            "outs": [],
                            "sync_info": {"on_update": [], "on_wait": grp},
                        })
                    nsplit += 1
                out.append(body)
            blk["instructions"] = out
    return nsplit


def _install_birfix():
    import concourse.bass_utils as bu
    import concourse.bass2jax as b2j

    if getattr(bu, "_waitsplit_installed", False):
        return
    orig = bu.compile_bir_kernel

    def patched(bir_json, tmpdir, neff_name="file.neff"):
        j = json.loads(bir_json)
        _split_waits_json(j)
        return orig(json.dumps(j).encode(), tmpdir, neff_name)

    bu.compile_bir_kernel = patched
    b2j.compile_bir_kernel = patched
    bu._waitsplit_installed = True


_install_birfix()

B, L, HID, H, D = 16, 512, 768, 12, 64
NS = 5
NCORES = 8
NB = B // NCORES          # batches per core
TOK = NB * L              # tokens per core
NJC = L // 128            # j-chunks per (b,h)
NG = HID // 128           # head-pair groups (6)
F32 = mybir.dt.float32
F16 = mybir.dt.float16
AF = mybir.ActivationFunctionType
OP = mybir.AluOpType

# struct -> mask-apply path. GPSIMD cannot read PSUM (and only supports
# plain TensorTensor ops), so:
#   'd' = DVE STT straight from PSUM;
#   'a' = ACT copy PSUM->SBUF f16 (the +ab bias rides the copy), then the
#         mask-multiply runs on Pool as tensor_mul from SBUF.
STT_ENG = ['d', 'd', 'd', 'a', 'a']
IDS = [0, 1]        # structs identity-accumulated on PE into the S0 bank
TREE = [2, 3, 4]    # summed by two DVE adds; joins s0 via a third identity

LAST_RESULT = None  # BassKernelResults of the most recent run (for test.py)


def _build_nc():
    nc = bass.Bass()

    # ---- DRAM I/O (per core) ----
    xt_h = nc.dram_tensor("xt", [HID, TOK], F16, kind="ExternalInput")
    wqt_h = nc.dram_tensor("wqt", [HID, HID], F16, kind="ExternalInput")
    wkt_h = nc.dram_tensor("wkt", [HID, HID], F16, kind="ExternalInput")
    wvt_h = nc.dram_tensor("wvt", [HID, HID], F16, kind="ExternalInput")
    bqt_h = nc.dram_tensor("bqt", [128, NG], F32, kind="ExternalInput")
    bkt_h = nc.dram_tensor("bkt", [128, NG], F32, kind="ExternalInput")
    bv_h = nc.dram_tensor("bv", [HID], F32, kind="ExternalInput")
    bilir_h = nc.dram_tensor("bilir", [D, H, NS, D], F16, kind="ExternalInput")
    absb_h = nc.dram_tensor("absb", [NS * H], F32, kind="ExternalInput")
    amt_h = nc.dram_tensor("amt", [128, NB * NJC], F32, kind="ExternalInput")
    maskt_h = nc.dram_tensor("maskt", [NB, NJC, 128, NS, L], F16, kind="ExternalInput")
    # [b, h, 65, i]: rows 0..63 = unnormalized ctx^T, row 64 = softmax denom
    out_h = nc.dram_tensor("out", [NB, H, 65, L], F16, kind="ExternalOutput")

    with tile.TileContext(nc) as tc:
        with tc.tile_pool(name="pers", bufs=1) as pers:
            # persistent SBUF tensors
            # paired q/k: group g holds heads (2g, 2g+1) on partition halves
            qt_t = [pers.tile([128, NG, L], F16, tag=f"qt{b}", name=f"qt{b}")
                    for b in range(NB)]
            kt_t = [pers.tile([128, NG, L], F16, tag=f"kt{b}", name=f"kt{b}")
                    for b in range(NB)]
            v_t = [pers.tile([128, H * 65], F16, tag=f"v{ic}", name=f"v{ic}")
                   for ic in range(NB * NJC)]
            # both partition halves hold the same [d, h, s, p] data so lhsT
            # can start at partition 0 or 64 to match the head's parity
            bilir_sb = pers.tile([128, H, NS, D], F16, tag="bilir")
            absb_sb = pers.tile([128, NS * H], F32, tag="absb")
            amt_sb = pers.tile([128, NB * NJC], F32, tag="amt")
            bqt_sb = pers.tile([128, NG], F32, tag="bqt")
            bkt_sb = pers.tile([128, NG], F32, tag="bkt")
            bv_sb = pers.tile([128, HID], F32, tag="bv")
            ident = pers.tile([128, 128], F16, tag="ident")

            make_identity(nc, ident[:, :])
            nc.sync.dma_start(out=amt_sb[:, :], in_=amt_h[:, :])
            nc.sync.dma_start(out=bqt_sb[:, :], in_=bqt_h[:, :])
            nc.sync.dma_start(out=bkt_sb[:, :], in_=bkt_h[:, :])
            ab_ap = absb_h[:]
            nc.gpsimd.dma_start(
                out=absb_sb[:, :],
                in_=bass.AP(tensor=ab_ap.tensor, offset=ab_ap.offset,
                            ap=[[0, 128], [1, NS * H]]),
            )
            bv_ap = bv_h[:]
            nc.gpsimd.dma_start(
                out=bv_sb[:, :],
                in_=bass.AP(tensor=bv_ap.tensor, offset=bv_ap.offset,
                            ap=[[0, 128], [1, HID]]),
            )
            # ones columns of v_ext: preset whole tile to 1.0; projection
            # evacuations overwrite the 64 value columns of each head slot.
            for ic in range(NB * NJC):
                nc.vector.memset(v_t[ic][:, :], 1.0)

            with (
                tc.tile_pool(name="stageb", bufs=1) as stb,
                tc.tile_pool(name="mpool", bufs=2) as mpool,
                tc.tile_pool(name="att", bufs=2) as att,
                tc.tile_pool(name="sc_ps", bufs=1, space="PSUM") as scp,
                tc.tile_pool(name="ctx_ps", bufs=2, space="PSUM") as ctxp,
            ):
                xt_sb = stb.tile([128, NG, TOK], F16, tag="xt")
                wq_sb = stb.tile([128, NG, HID], F16, tag="wq")
                wk_sb = stb.tile([128, NG, HID], F16, tag="wk")
                wv_sb = stb.tile([128, NG, HID], F16, tag="wv")
                # masks for batch b, loaded once, reused by all 12 heads
                mk_b = [mpool.tile([128, NJC, NS, L], F16, tag="mask",
                                   name=f"mask{b}") for b in range(NB)]

                # DMA queue ordered along the first iteration's critical
                # path: q's operands, then the first mask chunk, then the
                # rest interleaved.
                # batch-0's x half first: the first projection matmuls gate
                # on (wq chunk, xt-b0 chunk) pairs
                # DMA queue ordered along the first iterations' critical
                # path: q then k operands, bilir (for qs), first mask
                # chunk, then the rest.
                for hc in range(NG):
                    nc.sync.dma_start(out=wq_sb[:, hc, :], in_=wqt_h[hc * 128:(hc + 1) * 128, :])
                    nc.sync.dma_start(out=xt_sb[:, hc, 0:L], in_=xt_h[hc * 128:(hc + 1) * 128, 0:L])
                for hc in range(NG):
                    nc.sync.dma_start(out=wk_sb[:, hc, :], in_=wkt_h[hc * 128:(hc + 1) * 128, :])
                nc.sync.dma_start(out=bilir_sb[0:64, :, :, :], in_=bilir_h[:, :, :, :])
                nc.sync.dma_start(out=bilir_sb[64:128, :, :, :], in_=bilir_h[:, :, :, :])
                nc.sync.dma_start(out=mk_b[0][:, 0, :, :], in_=maskt_h[0, 0, :, :, :])
                for hc in range(NG):
                    nc.sync.dma_start(out=wv_sb[:, hc, :], in_=wvt_h[hc * 128:(hc + 1) * 128, :])
                for hc in range(NG):
                    nc.sync.dma_start(out=xt_sb[:, hc, L:TOK], in_=xt_h[hc * 128:(hc + 1) * 128, L:TOK])
                for jc in range(1, NJC):
                    nc.sync.dma_start(out=mk_b[0][:, jc, :, :], in_=maskt_h[0, jc, :, :, :])
                for jc in range(NJC):
                    nc.sync.dma_start(out=mk_b[1][:, jc, :, :], in_=maskt_h[1, jc, :, :, :])

                def emit_qk_unit(b, which, g):
                    # one head-pair group of the q or k projection
                    w_sb, t_sb, bias_sb = (
                        (wq_sb, qt_t[b], bqt_sb) if which == 'q'
                        else (wk_sb, kt_t[b], bkt_sb))
                    ps = scp.tile([128, L], F32, tag="s0", bufs=2,
                                  name=f"pj{which}{b}{g}")
                    for hc in range(NG):
                        nc.tensor.matmul(
                            ps[:, :],
                            lhsT=w_sb[:, hc, g * 128:(g + 1) * 128],
                            rhs=xt_sb[:, hc, b * L:(b + 1) * L],
                            start=(hc == 0), stop=(hc == NG - 1),
                        )
                    nc.scalar.activation(
                        t_sb[0:64, g, :], ps[0:64, :], AF.Identity,
                        bias=bias_sb[0:64, g:g + 1], scale=1.0,
                    )
                    nc.scalar.activation(
                        t_sb[64:128, g, :], ps[64:128, :], AF.Identity,
                        bias=bias_sb[64:128, g:g + 1], scale=1.0,
                    )

                def emit_v_unit(b, jc, ow, osz):
                    ic = b * NJC + jc
                    ps = scp.tile([128, 512], F32, tag="ss", bufs=4,
                                  name=f"vps{ic}{ow}")
                    for hc in range(NG):
                        nc.tensor.matmul(
                            ps[:, 0:osz],
                            lhsT=xt_sb[:, hc, ic * 128:(ic + 1) * 128],
                            rhs=wv_sb[:, hc, ow:ow + osz],
                            start=(hc == 0), stop=(hc == NG - 1),
                        )
                    h0 = ow // 64
                    nh = osz // 64
                    dst = v_t[ic][:, h0 * 65:(h0 + nh) * 65].rearrange(
                        "p (h e) -> p h e", e=65)[:, :, 0:64]
                    # DVE reads the PSUM into the slots; the host-broadcast
                    # bias is added there (Pool cannot read PSUM)
                    nc.vector.tensor_add(
                        dst,
                        ps[:, 0:osz].rearrange("p (h q) -> p h q", q=64),
                        bv_sb[:, ow:ow + osz].rearrange("p (h q) -> p h q", q=64),
                    )

                # minimal upfront (first three head-pairs + early V) so the
                # first attention iterations start ~30us sooner; the rest
                # drips through iteration slots ahead of use.
                emit_qk_unit(0, 'q', 0)
                emit_qk_unit(0, 'k', 0)
                def v_units(b, jc):
                    return [('v', b, jc, ow, osz)
                            for ow, osz in ((0, 512), (512, 256))]

                def qk_units(b, g):
                    return [('qk', b, w, g) for w in ('q', 'k')]

                # ordered so each unit lands before its first consumer
                drip_units = (
                    v_units(0, 0) + v_units(0, 1)
                    + v_units(0, 2) + v_units(0, 3)
                    + qk_units(0, 3) + qk_units(0, 4) + qk_units(0, 5)
                    + v_units(1, 0) + qk_units(1, 0) + v_units(1, 1)
                    + qk_units(1, 1) + v_units(1, 2) + qk_units(1, 2)
                    + qk_units(1, 3) + v_units(1, 3) + qk_units(1, 4)
                    + qk_units(1, 5)
                )

                def emit_drip(u):
                    if u[0] == 'qk':
                        emit_qk_unit(u[1], u[2], u[3])
                    else:
                        emit_v_unit(u[1], u[2], u[3], u[4])

                # ---- attention ----
                # Two heads (even/odd of each pair group) are software-
                # pipelined: their iteration bodies interleave so one
                # stream's matmul->STT->combine->exp chain latency hides
                # under the other stream's engine work.
                sgroups = [(0, 2), (2, 4), (4, 5)]

                qs_tiles = {}

                def emit_qs_unit(b, h, gi):
                    """One qs work unit: 1-2 matmuls + one PSUM evacuation.
                    Units are dripped through the schedule so the qs of pair
                    p+2 materializes during pair p's iterations."""
                    p0 = 64 * (h & 1)
                    g = h >> 1
                    qt_head = qt_t[b][p0:p0 + 64, g, :]
                    if (b, h) not in qs_tiles:
                        qs_tiles[(b, h)] = att.tile(
                            [128, NS, L], F16, tag="qs", bufs=6,
                            name=f"qs{b}_{h}")
                    qs_sb = qs_tiles[(b, h)]
                    s0i, s1i = sgroups[gi]
                    for s in range(s0i, s1i):
                        qs_ps = scp.tile([128, L], F32, tag="ss", bufs=4,
                                         name=f"qsps{b}{h}{s}")
                        nc.tensor.matmul(
                            qs_ps[p0:p0 + 64, :],
                            lhsT=bilir_sb[p0:p0 + 64, h, s, :],
                            rhs=qt_head,
                            start=True, stop=True,
                        )
                        if s == 4:  # spread evacuation load off ACT
                            nc.vector.tensor_copy(
                                qs_sb[p0:p0 + 64, s, :], qs_ps[p0:p0 + 64, :])
                        else:
                            nc.scalar.copy(
                                qs_sb[p0:p0 + 64, s, :], qs_ps[p0:p0 + 64, :])

                def emit_scores(b, h, jc, qs_sb):
                    """Phase 1: struct-score matmuls + mask STTs."""
                    p0 = 64 * (h & 1)
                    g = h >> 1
                    kt_j = kt_t[b][p0:p0 + 64, g, jc * 128:(jc + 1) * 128]
                    u_d = att.tile([128, 3, L], F16, tag="ud", bufs=4,
                                   name=f"ud{b}{h}{jc}")
                    u_p = att.tile([128, 2, L], F16, tag="up", bufs=4,
                                   name=f"up{b}{h}{jc}")
                    ssc = att.tile([128, 2, L], F16, tag="ssc", bufs=4,
                                   name=f"ssc{b}{h}{jc}")
                    nd = 0
                    npl = 0
                    uslot = {}
                    for s in range(NS):
                        ss = scp.tile([128, L], F32, tag="ss", bufs=4,
                                      name=f"ss{b}{h}{jc}{s}")
                        nc.tensor.matmul(
                            ss[:, :],
                            lhsT=kt_j,
                            rhs=qs_sb[p0:p0 + 64, s, :],
                            start=True, stop=True,
                        )
                        ab = absb_sb[:, s * H + h:s * H + h + 1]
                        mk = mk_b[b][:, jc, s, :]
                        if STT_ENG[s] == 'd':
                            dst = u_d[:, nd, :]
                            uslot[s] = dst
                            nd += 1
                            nc.vector.scalar_tensor_tensor(
                                dst, ss[:, :], ab, mk, OP.add, OP.mult)
                        else:
                            # ACT evacuates (ss + ab) to SBUF; Pool masks it
                            nc.scalar.activation(
                                ssc[:, npl, :], ss[:, :], AF.Identity,
                                bias=ab, scale=1.0)
                            dst = u_p[:, npl, :]
                            uslot[s] = dst
                            nc.gpsimd.tensor_mul(dst, ssc[:, npl, :], mk)
                            npl += 1
                    return uslot

                def combine_tree(b, h, jc, uslot):
                    """Structs 2-4 summed on DVE (f16 2x adds)."""
                    ta = att.tile([128, L], F16, tag="ta", bufs=3,
                                  name=f"ta{b}{h}{jc}")
                    nc.vector.tensor_add(ta[:, :], uslot[TREE[0]],
                                         uslot[TREE[1]])
                    tb = att.tile([128, L], F16, tag="tb", bufs=3,
                                  name=f"tb{b}{h}{jc}")
                    nc.vector.tensor_add(tb[:, :], ta[:, :], uslot[TREE[2]])
                    return tb

                def combine_pe(b, h, jc, uslot, tb, ctx_ps):
                    """S0 matmul, identity accumulates, exp, context."""
                    p0 = 64 * (h & 1)
                    g = h >> 1
                    qt_head = qt_t[b][p0:p0 + 64, g, :]
                    kt_j = kt_t[b][p0:p0 + 64, g, jc * 128:(jc + 1) * 128]
                    s0 = scp.tile([128, L], F32, tag="s0", bufs=2,
                                  name=f"s0_{b}_{h}_{jc}")
                    nc.tensor.matmul(s0[:, :], lhsT=kt_j, rhs=qt_head,
                                     start=True, stop=False,
                                     skip_group_check=True)
                    # structs 0-1 + the tree sum: PE identities into s0
                    adds = [uslot[s] for s in IDS] + [tb[:, :]]
                    for i, rhs in enumerate(adds):
                        nc.tensor.matmul(s0[:, :], lhsT=ident[:, :],
                                         rhs=rhs,
                                         start=False,
                                         stop=(i == len(adds) - 1),
                                         skip_group_check=True)
                    pr = att.tile([128, L], F16, tag="pr", bufs=4,
                                  name=f"pr{b}{h}{jc}")
                    nc.scalar.activation(
                        pr[:, :], s0[:, :], AF.Exp,
                        bias=amt_sb[:, b * NJC + jc:b * NJC + jc + 1],
                        scale=1.0,
                    )
                    nc.tensor.matmul(
                        ctx_ps[0:65, :],
                        lhsT=v_t[b * NJC + jc][:, h * 65:(h + 1) * 65],
                        rhs=pr[:, :],
                        start=(jc == 0), stop=(jc == NJC - 1),
                        skip_group_check=True,
                    )

                pairs = [(b, g) for b in range(NB) for g in range(NG)]

                def qs_units(pi):
                    if pi >= len(pairs):
                        return []
                    b_, g_ = pairs[pi]
                    return [(b_, 2 * g_ + hi, gi)
                            for hi in range(2) for gi in range(3)]

                # cold-start sequenced along the first pair's critical path:
                # pair 0's qs right after its projections, then the next two
                # head-pair projections and pair 1's qs; pair p+2's qs drips
                # through pair p's iteration slots.
                for unit in qs_units(0):
                    emit_qs_unit(*unit)
                emit_qk_unit(0, 'q', 1)
                emit_qk_unit(0, 'k', 1)
                for unit in qs_units(1):
                    emit_qs_unit(*unit)
                emit_qk_unit(0, 'q', 2)
                emit_qk_unit(0, 'k', 2)
                for pi, (b, g) in enumerate(pairs):
                    heads = (2 * g, 2 * g + 1)
                    qs_pair = [qs_tiles[(b, h)] for h in heads]
                    drip = list(qs_units(pi + 2))
                    ctx_pair = [ctxp.tile([65, L], F32, tag="ctx",
                                          name=f"ctx{b}_{h}")
                                for h in heads]
                    for jc in range(NJC):
                        us = []
                        for hi, h in enumerate(heads):
                            us.append(emit_scores(b, h, jc, qs_pair[hi]))
                        if jc >= 1:
                            for _ in range(2):
                                if drip:
                                    emit_qs_unit(*drip.pop(0))
                        for _ in range(2 if pi < 3 else 1):
                            if drip_units:
                                emit_drip(drip_units.pop(0))
                        # both heads' DVE trees first, then their PE/ACT
                        # chains, so each engine sees back-to-back work
                        tbs = [combine_tree(b, h, jc, us[hi])
                               for hi, h in enumerate(heads)]
                        for hi, h in enumerate(heads):
                            combine_pe(b, h, jc, us[hi], tbs[hi],
                                       ctx_pair[hi])
                    # raw ctx^T + denominator row to DRAM via an SBUF
                    # bounce; the host transposes and normalizes.
                    for hi, h in enumerate(heads):
                        ct = att.tile([65, L], F16, tag="ct", bufs=4,
                                      name=f"ct{b}_{h}")
                        nc.scalar.copy(ct[:, :], ctx_pair[hi][0:65, :])
                        nc.sync.dma_start(out=out_h[b, h, :, :], in_=ct[:, :])
    return nc


_NC = None


def _get_nc():
    global _NC
    if _NC is None:
        _NC = _build_nc()
    return _NC


def _prep_inputs(inputs):
    hs = np.asarray(inputs["hidden_states"], np.float32)
    am = np.asarray(inputs["attention_mask"], np.float32).reshape(B, L)
    sm = np.asarray(inputs["structure_mask"], np.float32)
    Wq = np.asarray(inputs["Wq"], np.float32)
    Wk = np.asarray(inputs["Wk"], np.float32)
    Wv = np.asarray(inputs["Wv"], np.float32)
    bq = np.asarray(inputs["bq"], np.float32)
    bk = np.asarray(inputs["bk"], np.float32)
    bv = np.asarray(inputs["bv"], np.float32)
    bili = np.asarray(inputs["bili"], np.float32)
    absb = np.asarray(inputs["abs_bias"], np.float32)

    sc = np.float32(1.0 / np.sqrt(D))  # folded into the whole q side
    shared = {
        "wqt": np.ascontiguousarray(Wq.T * sc).astype(np.float16),
        "wkt": np.ascontiguousarray(Wk.T).astype(np.float16),
        "wvt": np.ascontiguousarray(Wv.T).astype(np.float16),
        "bqt": np.ascontiguousarray((bq * sc).reshape(NG, 128).T),
        "bkt": np.ascontiguousarray(bk.reshape(NG, 128).T),
        "bv": bv,
        # [d, h, s, p] so lhsT slice [:, h, s:s+2, :] pairs two structs
        "bilir": np.ascontiguousarray(
            bili.transpose(2, 1, 0, 3)).astype(np.float16),
        "absb": np.ascontiguousarray(absb.reshape(NS * H) * sc),
    }
    in_maps = []
    for c in range(NCORES):
        b0 = c * NB
        x = hs[b0:b0 + NB].reshape(TOK, HID)
        amc = am[b0:b0 + NB]  # [NB, L]
        # -10: constant logit shift (softmax-invariant) keeping exp() and the
        # row sums inside fp16 range without a max-reduction pass.
        amt = np.ascontiguousarray(
            amc.reshape(NB, NJC, 128).transpose(2, 0, 1)).reshape(128, NB * NJC) - 10.0
        mk = sm[:, b0:b0 + NB, 0]  # [NS, NB, L(i), L(j)]
        mkt = np.ascontiguousarray(mk.transpose(1, 3, 0, 2))  # [NB, j, NS, i]
        mkt = mkt.reshape(NB, NJC, 128, NS, L).astype(np.float16)
        in_maps.append(dict(
            xt=np.ascontiguousarray(x.T).astype(np.float16), amt=amt, maskt=mkt, **shared))
    return in_maps


def kernel(**inputs):
    global LAST_RESULT
    nc = _get_nc()
    in_maps = _prep_inputs(inputs)
    import os
    trace = bool(os.environ.get("BASS_TRACE"))
    try:
        LAST_RESULT = run_bass_kernel_spmd(
            nc, in_maps, core_ids=list(range(NCORES)), trace=trace)
    except ModuleNotFoundError:
        # axon NTFF profile hook unavailable in this environment
        LAST_RESULT = run_bass_kernel_spmd(
            nc, in_maps, core_ids=list(range(NCORES)), trace=False)
    outs = np.stack([r["out"] for r in LAST_RESULT.results]).astype(np.float32)
    den = outs[:, :, :, 64:65, :]  # [8, NB, H, 1, L]
    vals = outs[:, :, :, 0:64, :] / den
    # [c, b, h, d, i] -> [c, b, i, h, d] -> [B, L, HID]
    return np.ascontiguousarray(vals.transpose(0, 1, 4, 2, 3)).reshape(B, L, HID)
